# revision 52
# baseline (speedup 1.0000x reference)
"""Trainium2 Bass kernel for nn_Densenet_with_skip (gauss blur -> sobel ->
angle-binned 8-direction NMS -> gate).

Reformulation (same math as the validated baseline):
  b  = gauss5x5(x)                      (zero pad; separable, rank-1)
  gx/gy via composed 7-tap band matmuls (replicate pad on b)
  bin: m0 = (t1*|gx| >= |gy|)  -> horizontal pair
       m2 = (t1*|gy| >  |gx|)  -> vertical pair
       else diag: gx*gy<0 (<=> |gx+gy| < sqrt2*|gx|) -> anti-diag pair
  out = x * relu(cen*b + v*max(pair))

v2 engine-balance rewrite (vs the STT-heavy baseline):
  - 3 deduped weight matrices (gauss is symmetric: Bv==Bh; Sv==Sh; Dv==Dh),
    shared between both matmul passes; scales folded into the pass-1/pass-2
    PSUM->SBUF extract `scale` so every elementwise compare is a plain
    fp16 tensor_tensor (2x DVE mode) or tensor_scalar (4x DVE mode).
  - pass-2 accumulates [b | gx | gy] into ONE 3-bank PSUM tile per row-tile
    so a single Abs-activation extracts S/A/Y together (b >= 0 so Abs is a
    no-op on the b slice); |gx+gy| rides the same psum via 4 extra matmuls.
  - pair maxes run on the idle Pool(gpsimd) engine; masks + selection
    (copy_predicated) + scalar chain stay on DVE; extracts on Act.
  - per-image batched DMAs (x load / out store); Up/Dn row-shifted copies
    of S stay on the DMA engines.
"""

import sys

import numpy as np

sys.path.insert(0, "/opt/trn_rl_repo")

import concourse.bacc as bacc
import concourse.mybir as mybir
from concourse import tile
from concourse.bass_utils import run_bass_kernel_spmd

N = 512
B_TOTAL = 32
N_CORES = 8
B_CORE = B_TOTAL // N_CORES  # 4 images per core
NCHUNK = N // 128  # 4

F16 = mybir.dt.float16
F32 = mybir.dt.float32
U16 = mybir.dt.uint16

T1 = float(np.tan(np.pi / 8))  # tan(22.5 deg)
RT2 = float(np.sqrt(2.0))

ALU = mybir.AluOpType
AF = mybir.ActivationFunctionType


def _band_ranges(halo):
    out = []
    for r in range(NCHUNK):
        lo = max(0, 128 * r - halo)
        hi = min(N, 128 * r + 128 + halo)
        out.append((lo, hi))
    return out


R7 = _band_ranges(3)


def _banded_mm(nc, psum_ap, lhsT_sl, w_chunks, start_fresh=True, stop=True,
               out_off=0):
    """Accumulate sum_r lhsT_r.T @ W_r[:, band] into psum[:, out_off + band]
    with PSUM fresh/covered split handling (each matmul touches either
    all-fresh or all-covered columns)."""
    covered = 0
    n = NCHUNK
    for r in range(n):
        lo, hi = R7[r]
        first = r == 0
        last = r == n - 1
        if not first and lo < covered:
            nc.tensor.matmul(
                psum_ap[:, out_off + lo : out_off + covered],
                lhsT_sl[r],
                w_chunks[:, r, lo:covered],
                start=False,
                stop=False,
            )
            lo = covered
        nc.tensor.matmul(
            psum_ap[:, out_off + lo : out_off + hi],
            lhsT_sl[r],
            w_chunks[:, r, lo:hi],
            start=first and start_fresh,
            stop=last and stop,
        )
        covered = hi


def _accum_mm(nc, psum_ap, lhsT_sl, w_chunks, stop=True, out_off=0,
              skip_group_check=False):
    """Accumulate onto an already-covered psum range (no splits needed)."""
    for r in range(NCHUNK):
        lo, hi = R7[r]
        nc.tensor.matmul(
            psum_ap[:, out_off + lo : out_off + hi],
            lhsT_sl[r],
            w_chunks[:, r, lo:hi],
            start=False,
            stop=stop and (r == NCHUNK - 1),
            skip_group_check=skip_group_check,
        )


def build_nc(cen, v):
    s = -v  # S = s*b ; z = (cen/s)*S - sel
    zs = cen / s

    nc = bacc.Bacc("TRN2", target_bir_lowering=False, debug=False)

    x_d = nc.dram_tensor("x", [B_CORE * N, N], F32, kind="ExternalInput").ap()
    # three deduped band matrices as one cat tensor [3, 512, 512] fp16:
    # plane 0: G (gauss 5-tap), plane 1: 2*M_sm (Bv@Sm), plane 2: 2*M_df
    wcat_d = nc.dram_tensor("wcat", [3 * N, N], F16, kind="ExternalInput").ap()
    out_d = nc.dram_tensor("out", [B_CORE * N, N], F32, kind="ExternalOutput").ap()

    W2 = N + 2  # padded row width for col-shifted access

    with tile.TileContext(nc) as tc:
        with (
            tc.tile_pool(name="wpool", bufs=1) as wpool,
            tc.tile_pool(name="xpool", bufs=1) as xpool,
            tc.tile_pool(name="tT", bufs=2) as tTpool,
            tc.tile_pool(name="post", bufs=2) as post,
            tc.tile_pool(name="post1", bufs=1) as post1,
            tc.tile_pool(name="outp", bufs=2) as outp,
            tc.tile_pool(name="psum", bufs=2, space="PSUM") as psum,
        ):
            zrow = wpool.tile([1, W2], F16, tag="zrow")
            nc.vector.memset(zrow[:], 0.0)

            # --- load x as fp16 (DMA cast); quad layout [128, 4*512]:
            # image 0 alone (so compute starts early), images 1-3 batched
            x0t = xpool.tile([128, NCHUNK * N], F16, tag="xh_0")
            nc.gpsimd.dma_start(
                out=x0t[:].rearrange("p (r w) -> p r w", w=N),
                in_=x_d[0:N, :].rearrange("(r p) w -> p r w", p=128),
            )
            xh = [x0t[:], None, None, None]

            # --- weight cat [128, 3, 4, 512]; one DMA per row-chunk so the
            # first pass-1 matmuls can start early
            wt = wpool.tile([128, 3 * NCHUNK * N], F16, tag="wcat")
            w4 = wt[:].rearrange("p (k r w) -> p k r w", k=3, w=N)
            wsrc = wcat_d.rearrange("(k r p) w -> p r k w", k=3, p=128)
            for r in range(NCHUNK):
                nc.sync.dma_start(out=w4[:, :, r, :], in_=wsrc[:, r])
            w_sb = {"g": w4[:, 0], "msm": w4[:, 1], "mdf": w4[:, 2]}

            # images 1-3 in one DMA (overlaps image-0 compute)
            x13 = xpool.tile([128, 3 * NCHUNK * N], F16, tag="xh_13")
            nc.gpsimd.dma_start(
                out=x13[:].rearrange("p (i r w) -> p i r w", i=3, w=N),
                in_=x_d[N:, :].rearrange("(i r p) w -> p i r w", p=128, i=3),
            )
            for i in range(1, B_CORE):
                xh[i] = x13[:, (i - 1) * NCHUNK * N : i * NCHUNK * N]

            for i in range(B_CORE):
                # ---------- pass 1 (vertical), all 3 kernels per matmul ------
                # psum planes: [G | 2*M_sm | 2*M_df] applied down the rows;
                # one plain Copy extract per col-chunk (scales live in W).
                tTc = []
                for c in range(NCHUNK):
                    pv = psum.tile([128, 3 * N], F32, tag="cat")
                    pv3 = pv[:].rearrange("p (k w) -> p k w", w=N)
                    lhsT = [
                        xh[i][:, N * r + 128 * c : N * r + 128 * (c + 1)]
                        for r in range(NCHUNK)
                    ]
                    # one matmul per kernel-plane: a psum AP must stay inside
                    # a single 2KB psum bank
                    covered = 0
                    for r in range(NCHUNK):
                        lo, hi = R7[r]
                        first = r == 0
                        if not first and lo < covered:
                            for k in range(3):
                                nc.tensor.matmul(
                                    pv3[:, k, lo:covered], lhsT[r],
                                    w4[:, k, r, lo:covered],
                                    start=False, stop=False,
                                )
                            lo = covered
                        for k in range(3):
                            nc.tensor.matmul(
                                pv3[:, k, lo:hi], lhsT[r], w4[:, k, r, lo:hi],
                                start=first, stop=(r == NCHUNK - 1),
                            )
                        covered = hi
                    st = tTpool.tile([128, 3 * N], F16, tag=f"tT_{c}")
                    if i == 0 and c % 2 == 1:
                        # during pipeline fill DVE is idle: split image 0's
                        # extracts between Act and DVE to start sooner
                        nc.vector.tensor_scalar(out=st[:], in0=pv[:],
                                                scalar1=1.0, op0=ALU.mult,
                                                scalar2=None)
                    else:
                        nc.scalar.activation(st[:], pv[:], AF.Copy)
                    tTc.append(st)
                # pass-2 stationary slices: tTc[c][:, k*512 + rows]
                tT = {
                    k: [tTc[c][:, kk * N : (kk + 1) * N] for c in range(NCHUNK)]
                    for kk, k in enumerate(("b", "x", "y"))
                }

                # ---------- pass 2 (horizontal) into cat psum [128, 3*512] ----
                # slice 0: b ; slice 1: 2*gx (then 2*(gx+gy)) ; slice 2: 2*gy
                # Eq holds the Abs-extract [S | A | Y] in padded quad layout
                # [128, 3, 4, 514]: S = s*b, A = 2s|gx|, Y = 2s|gy|
                Eq = post.tile([128, 3 * NCHUNK * W2], F16, tag="Eq")
                E4 = Eq[:].rearrange("p (k q w) -> p k q w", k=3, w=W2)
                # zero the S-plane column pads (cols 0 and 513 of each q)
                nc.gpsimd.memset(E4[:, 0, :, 0:1], 0.0)
                nc.gpsimd.memset(E4[:, 0, :, N + 1 : N + 2], 0.0)
                Pq = post.tile([128, NCHUNK * N], F16, tag="Pq")

                for rt in range(NCHUNK):
                    row0 = 128 * rt
                    p2 = psum.tile([128, 3 * N], F32, tag="cat")

                    def sl(key, c):
                        return tT[key][c][:, row0 : row0 + 128]

                    _banded_mm(nc, p2, [sl("b", c) for c in range(NCHUNK)],
                               w_sb["g"], stop=True, out_off=0)
                    _banded_mm(nc, p2, [sl("x", c) for c in range(NCHUNK)],
                               w_sb["mdf"], stop=True, out_off=N)
                    _banded_mm(nc, p2, [sl("y", c) for c in range(NCHUNK)],
                               w_sb["msm"], stop=True, out_off=2 * N)

                    # one Abs extract of all 3 slices; scale s gives
                    # S = s|b| = s*b, A = 2s|gx|, Y = 2s|gy|
                    nc.scalar.activation(
                        E4[:, :, rt, 1 : N + 1],
                        p2[:].rearrange("p (k w) -> p k w", w=N),
                        AF.Abs,
                        scale=s,
                    )
                    # u: gx-slice += 2*gy -> 2*(gx+gy); P = (s/sqrt2)*|2u|
                    #   mneg test: |u| < sqrt2|gx| <=> P < A
                    _accum_mm(nc, p2, [sl("y", c) for c in range(NCHUNK)],
                              w_sb["msm"], stop=True, out_off=N,
                              skip_group_check=True)
                    nc.scalar.activation(
                        Pq[:, rt * N : (rt + 1) * N],
                        p2[:, N : 2 * N],
                        AF.Abs,
                        scale=s / RT2,
                    )

                # ---------- Up/Dn shifted copies of the S plane via DMA ------
                # (issued per half so the select chain starts before the whole
                # image's pass-2 finishes)
                Upq = post.tile([128, NCHUNK * W2], F16, tag="Up")
                Dnq = post.tile([128, NCHUNK * W2], F16, tag="Dn")
                S3 = E4[:, 0]  # [128, 4, 514]
                U3 = Upq[:].rearrange("p (q w) -> p q w", w=W2)
                D3 = Dnq[:].rearrange("p (q w) -> p q w", w=W2)
                # boundary zeros (image edge rows): Up[127, 3] = 0; Dn[0, 0] = 0
                # (engine ops need quadrant-aligned start partitions; the
                # partition-127 sliver goes via DMA from the zero row)
                nc.sync.dma_start(
                    out=U3[127:128, 3:4, :],
                    in_=zrow[:].rearrange("p (q w) -> p q w", w=W2),
                )
                nc.gpsimd.memset(D3[0:1, 0:1, :], 0.0)

                A3 = E4[:, 1, :, 1 : N + 1]   # 4s|gx|
                Y3v = E4[:, 2, :, 1 : N + 1]  # 4s|gy|
                P3 = Pq[:].rearrange("p (q w) -> p q w", w=N)

                def qt(pool, tag, dt=F16):
                    t = pool.tile([128, NCHUNK * N], dt, tag=tag)
                    return t, t[:].rearrange("p (q w) -> p q w", w=N)

                Yloq, Ylo3 = qt(post1, "Ylo")  # Y/t1  (m0 = A >= Ylo)
                Yhiq, Yhi3 = qt(post1, "Yhi")  # Y*t1  (m2 = Yhi > A)
                m0q, m03 = qt(post1, "m0", U16)
                m2q, m23 = qt(post1, "m2", U16)
                mnq, mn3 = qt(post1, "mn", U16)
                selq, sel3 = qt(post, "sel")
                pmAq, pmA3 = qt(post, "pmA")
                pmVq, pmV3 = qt(post, "pmV")
                pmHq, pmH3 = qt(post, "pmH")
                Cq, C3 = qt(post1, "C")
                zq, z3 = qt(post1, "z")
                zrq, zr3 = qt(post1, "zr")
                oq = outp.tile([128, NCHUNK * N], F16, tag="o")
                o3 = oq[:].rearrange("p (q w) -> p q w", w=N)
                x3 = xh[i].rearrange("p (q w) -> p q w", w=N)

                TTv = nc.vector.tensor_tensor
                TTp = nc.gpsimd.tensor_tensor
                TSv = nc.vector.tensor_scalar

                # the select stage runs per image-half: shorter dependency
                # chains pipeline better across engines
                parts = [(0, 2), (2, 4)]
                for q0, q1 in parts:
                    hs = slice(q0, q1)
                    # Up/Dn: main shift + wrap rows
                    nc.sync.dma_start(out=U3[0:127, hs, :],
                                      in_=S3[1:128, hs, :])
                    nc.sync.dma_start(out=D3[1:128, hs, :],
                                      in_=S3[0:127, hs, :])
                    # Up[127, q] = S[0, q+1] (q < 3)
                    qh = min(q1, 3)
                    if qh > q0:
                        nc.sync.dma_start(out=U3[127:128, q0:qh, :],
                                          in_=S3[0:1, q0 + 1 : qh + 1, :])
                    # Dn[0, q] = S[127, q-1] (q > 0)
                    ql = max(q0, 1)
                    if q1 > ql:
                        nc.sync.dma_start(out=D3[0:1, ql:q1, :],
                                          in_=S3[127:128, ql - 1 : q1 - 1, :])

                    # scaled |gy| copies (DVE tensor_scalar, 4x mode)
                    TSv(out=Ylo3[:, hs], in0=Y3v[:, hs], scalar1=1.0 / T1,
                        op0=ALU.mult, scalar2=None)
                    TSv(out=Yhi3[:, hs], in0=Y3v[:, hs], scalar1=T1,
                        op0=ALU.mult, scalar2=None)
                    # masks (DVE tensor_tensor, 2x mode)
                    TTv(out=m03[:, hs], in0=A3[:, hs], in1=Ylo3[:, hs],
                        op=ALU.is_ge)
                    TTv(out=m23[:, hs], in0=Yhi3[:, hs], in1=A3[:, hs],
                        op=ALU.is_gt)
                    TTv(out=mn3[:, hs], in0=A3[:, hs], in1=P3[:, hs],
                        op=ALU.is_gt)
                    # pair maxes (DVE: Pool's GPSIMD has no max kernel)
                    TTv(out=sel3[:, hs], in0=D3[:, hs, 0:N],
                        in1=U3[:, hs, 2 : N + 2], op=ALU.max)  # main diag
                    TTv(out=pmA3[:, hs], in0=D3[:, hs, 2 : N + 2],
                        in1=U3[:, hs, 0:N], op=ALU.max)  # anti diag
                    TTv(out=pmV3[:, hs], in0=D3[:, hs, 1 : N + 1],
                        in1=U3[:, hs, 1 : N + 1], op=ALU.max)  # vertical
                    TTv(out=pmH3[:, hs], in0=S3[:, hs, 0:N],
                        in1=S3[:, hs, 2 : N + 2], op=ALU.max)  # horizontal
                    # selection cascade (DVE copy_predicated)
                    nc.vector.copy_predicated(sel3[:, hs], mn3[:, hs],
                                              pmA3[:, hs])
                    nc.vector.copy_predicated(sel3[:, hs], m23[:, hs],
                                              pmV3[:, hs])
                    nc.vector.copy_predicated(sel3[:, hs], m03[:, hs],
                                              pmH3[:, hs])
                    # z = zs*S - sel ; out = relu(z) * x
                    # Pool runs this chain (ts_mult/tt_sub/ts_max/tt_mult are
                    # the GPSIMD kernels that exist); last image on DVE for a
                    # short drain tail.
                    TSx = TSv if i == B_CORE - 1 else nc.gpsimd.tensor_scalar
                    TTc = TTv if i == B_CORE - 1 else TTp
                    TSx(out=C3[:, hs], in0=S3[:, hs, 1 : N + 1], scalar1=zs,
                        op0=ALU.mult, scalar2=None)
                    TTc(out=z3[:, hs], in0=C3[:, hs], in1=sel3[:, hs],
                        op=ALU.subtract)
                    TSx(out=zr3[:, hs], in0=z3[:, hs], scalar1=0.0,
                        op0=ALU.max, scalar2=None)
                    TTc(out=o3[:, hs], in0=zr3[:, hs], in1=x3[:, hs],
                        op=ALU.mult)
                    # store this part (fp16 -> fp32 cast)
                    nc.gpsimd.dma_start(
                        out=out_d[i * N + 128 * q0 : i * N + 128 * q1, :]
                        .rearrange("(r p) w -> p r w", p=128),
                        in_=o3[:, hs],
                    )

    nc.compile()
    return nc


# ---------------------------------------------------------------------------
# host side
# ---------------------------------------------------------------------------

def _make_band(weights, offsets, pad):
    M = np.zeros((N, N), dtype=np.float64)
    for w, o in zip(weights, offsets):
        idx = np.arange(N)
        src = idx + o
        if pad == "replicate":
            np.add.at(M, (np.clip(src, 0, N - 1), idx), w)
        else:
            ok = (src >= 0) & (src < N)
            np.add.at(M, (src[ok], idx[ok]), w)
    return M


def _host_weights(gauss_kernel):
    gk = np.asarray(gauss_kernel, dtype=np.float64)[0, 0]
    U, sv, Vt = np.linalg.svd(gk)
    assert sv[1] < 1e-5 * sv[0], "gauss kernel not rank-1 separable"
    wv = U[:, 0] * np.sqrt(sv[0])
    wh = Vt[0] * np.sqrt(sv[0])
    if wv.sum() < 0:
        wv, wh = -wv, -wh
    # symmetry of the gauss kernel => Bv == Bh (same 1-D factor both ways)
    assert np.allclose(wv, wh, atol=1e-12), "gauss kernel not symmetric"
    o5 = [-2, -1, 0, 1, 2]
    o3 = [-1, 0, 1]
    G = _make_band(wv, o5, "zero")
    Sm = _make_band([1, 2, 1], o3, "replicate")
    Df = _make_band([-1, 0, 1], o3, "replicate")
    wcat = np.concatenate([G, 2.0 * (G @ Sm), 2.0 * (G @ Df)], axis=0)
    return {"wcat": np.ascontiguousarray(wcat, dtype=np.float16)}



_NC_CACHE = {}
LAST_RESULT = None


def kernel(reconst, gauss_kernel, nms_kernel):
    nk = np.asarray(nms_kernel, dtype=np.float64)
    cen = float(nk[0, 0, 1, 1])
    v = float(nk[0, 0, 1, 2])
    # verify nms kernel structure: center + single tap v per direction
    pos = [(1, 2), (2, 2), (2, 1), (2, 0), (1, 0), (0, 0), (0, 1), (0, 2)]
    for d, (r, c) in enumerate(pos):
        k = nk[d, 0].copy()
        assert abs(k[1, 1] - cen) < 1e-6 and abs(k[r, c] - v) < 1e-6
        k[1, 1] = 0.0
        k[r, c] = 0.0
        assert np.abs(k).max() < 1e-7
    assert v < 0

    key = (round(cen, 9), round(v, 9))
    if key not in _NC_CACHE:
        _NC_CACHE[key] = build_nc(cen, v)
    nc = _NC_CACHE[key]

    w = _host_weights(gauss_kernel)
    x = np.asarray(reconst, dtype=np.float32).reshape(B_TOTAL, N, N)
    in_maps = []
    for core in range(N_CORES):
        m = {"x": np.ascontiguousarray(
            x[core * B_CORE : (core + 1) * B_CORE].reshape(B_CORE * N, N)
        )}
        m.update(w)
        in_maps.append(m)

    res = run_bass_kernel_spmd(nc, in_maps, core_ids=list(range(N_CORES)))
    global LAST_RESULT
    LAST_RESULT = res
    out = np.concatenate(
        [r["out"].reshape(B_CORE, 1, N, N) for r in res.results], axis=0
    )
    return out.astype(np.float32)


# revision 75
# speedup vs baseline: 1.0305x; 1.0305x over previous
"""Trainium2 Bass kernel for nn_Densenet_with_skip (gauss blur -> sobel ->
angle-binned 8-direction NMS -> gate).

Reformulation (same math as the validated baseline):
  b  = gauss5x5(x)                      (zero pad; separable, rank-1)
  gx/gy via composed 7-tap band matmuls (replicate pad on b)
  bin: m0 = (t1*|gx| >= |gy|)  -> horizontal pair
       m2 = (t1*|gy| >  |gx|)  -> vertical pair
       else diag: gx*gy<0 (<=> |gx+gy| < sqrt2*|gx|) -> anti-diag pair
  out = x * relu(cen*b + v*max(pair))

v2 engine-balance rewrite (vs the STT-heavy baseline):
  - 3 deduped weight matrices (gauss is symmetric: Bv==Bh; Sv==Sh; Dv==Dh),
    shared between both matmul passes; scales folded into the pass-1/pass-2
    PSUM->SBUF extract `scale` so every elementwise compare is a plain
    fp16 tensor_tensor (2x DVE mode) or tensor_scalar (4x DVE mode).
  - pass-2 accumulates [b | gx | gy] into ONE 3-bank PSUM tile per row-tile
    so a single Abs-activation extracts S/A/Y together (b >= 0 so Abs is a
    no-op on the b slice); |gx+gy| rides the same psum via 4 extra matmuls.
  - engine split: masks/pair-maxes/selection (copy_predicated, u16 masks)
    on DVE; the zs*S-sel / relu / gate chain on Pool (only add/sub/mult/
    ts_mult/ts_max exist as GPSIMD kernels; max/is_ge/STT do not compile);
    PSUM->SBUF extracts on Act; last image's chain on DVE for a short tail.
  - per-image-half select stage for cross-engine pipelining; batched DMAs
    (x load / out store); Up/Dn row-shifted copies of S on the DMA engines.
"""

import sys

import numpy as np

sys.path.insert(0, "/opt/trn_rl_repo")

import concourse.bacc as bacc
import concourse.mybir as mybir
from concourse import tile
from concourse.bass_utils import run_bass_kernel_spmd

N = 512
B_TOTAL = 32
N_CORES = 8
B_CORE = B_TOTAL // N_CORES  # 4 images per core
NCHUNK = N // 128  # 4

F16 = mybir.dt.float16
F32 = mybir.dt.float32
U16 = mybir.dt.uint16

T1 = float(np.tan(np.pi / 8))  # tan(22.5 deg)
RT2 = float(np.sqrt(2.0))

ALU = mybir.AluOpType
AF = mybir.ActivationFunctionType


def _band_ranges(halo):
    out = []
    for r in range(NCHUNK):
        lo = max(0, 128 * r - halo)
        hi = min(N, 128 * r + 128 + halo)
        out.append((lo, hi))
    return out


R7 = _band_ranges(3)
WSTRIP = 136  # compact W strip width (>= max band range width 134)


def _banded_mm(nc, psum_ap, lhsT_sl, w_chunks, start_fresh=True, stop=True,
               out_off=0):
    """Accumulate sum_r lhsT_r.T @ W_r[:, band] into psum[:, out_off + band]
    with PSUM fresh/covered split handling (each matmul touches either
    all-fresh or all-covered columns)."""
    covered = 0
    n = NCHUNK
    for r in range(n):
        lo, hi = R7[r]
        first = r == 0
        last = r == n - 1
        base = R7[r][0]
        if not first and lo < covered:
            nc.tensor.matmul(
                psum_ap[:, out_off + lo : out_off + covered],
                lhsT_sl[r],
                w_chunks[:, r, lo - base : covered - base],
                start=False,
                stop=False,
            )
            lo = covered
        nc.tensor.matmul(
            psum_ap[:, out_off + lo : out_off + hi],
            lhsT_sl[r],
            w_chunks[:, r, lo - base : hi - base],
            start=first and start_fresh,
            stop=last and stop,
        )
        covered = hi


def _accum_mm(nc, psum_ap, lhsT_sl, w_chunks, stop=True, out_off=0,
              skip_group_check=False):
    """Accumulate onto an already-covered psum range (no splits needed)."""
    for r in range(NCHUNK):
        lo, hi = R7[r]
        base = R7[r][0]
        nc.tensor.matmul(
            psum_ap[:, out_off + lo : out_off + hi],
            lhsT_sl[r],
            w_chunks[:, r, lo - base : hi - base],
            start=False,
            stop=stop and (r == NCHUNK - 1),
            skip_group_check=skip_group_check,
        )


def build_nc(cen, v):
    s = -v  # S = s*b ; z = (cen/s)*S - sel
    zs = cen / s

    nc = bacc.Bacc("TRN2", target_bir_lowering=False, debug=False)

    x_d = nc.dram_tensor("x", [B_CORE * N, N], F32, kind="ExternalInput").ap()
    # three deduped band matrices as one cat tensor [3, 512, 512] fp16:
    # plane 0: G (gauss 5-tap), plane 1: 2*M_sm (Bv@Sm), plane 2: 2*M_df
    wcat_d = nc.dram_tensor(
        "wcat", [3 * NCHUNK * 128, WSTRIP], F16, kind="ExternalInput"
    ).ap()
    out_d = nc.dram_tensor("out", [B_CORE * N, N], F32, kind="ExternalOutput").ap()

    W2 = N + 2  # padded row width for col-shifted access

    with tile.TileContext(nc) as tc:
        with (
            tc.tile_pool(name="wpool", bufs=1) as wpool,
            tc.tile_pool(name="xpool", bufs=1) as xpool,
            tc.tile_pool(name="tT", bufs=2) as tTpool,
            tc.tile_pool(name="post", bufs=2) as post,
            tc.tile_pool(name="post1", bufs=1) as post1,
            tc.tile_pool(name="outp", bufs=2) as outp,
            tc.tile_pool(name="psum", bufs=2, space="PSUM") as psum,
        ):
            zrow = wpool.tile([1, W2], F16, tag="zrow")
            nc.vector.memset(zrow[:], 0.0)

            # --- load x as fp16 (DMA cast); quad layout [128, 4*512]:
            # image 0 alone (so compute starts early), images 1-3 batched
            x0t = xpool.tile([128, NCHUNK * N], F16, tag="xh_0")
            nc.gpsimd.dma_start(
                out=x0t[:].rearrange("p (r w) -> p r w", w=N),
                in_=x_d[0:N, :].rearrange("(r p) w -> p r w", p=128),
            )
            xh = [x0t[:], None, None, None]

            # --- weight cat [128, 3, 4, 512]; one DMA per row-chunk so the
            # first pass-1 matmuls can start early
            wt = wpool.tile([128, 3 * NCHUNK * WSTRIP], F16, tag="wcat")
            w4 = wt[:].rearrange("p (k r w) -> p k r w", k=3, w=WSTRIP)
            wsrc = wcat_d.rearrange("(k r p) w -> p r k w", k=3, p=128)
            for r in range(NCHUNK):
                nc.sync.dma_start(out=w4[:, :, r, :], in_=wsrc[:, r])
            w_sb = {"g": w4[:, 0], "msm": w4[:, 1], "mdf": w4[:, 2]}

            # images 1-3 in one DMA (overlaps image-0 compute)
            x13 = xpool.tile([128, 3 * NCHUNK * N], F16, tag="xh_13")
            nc.gpsimd.dma_start(
                out=x13[:].rearrange("p (i r w) -> p i r w", i=3, w=N),
                in_=x_d[N:, :].rearrange("(i r p) w -> p i r w", p=128, i=3),
            )
            for i in range(1, B_CORE):
                xh[i] = x13[:, (i - 1) * NCHUNK * N : i * NCHUNK * N]

            for i in range(B_CORE):
                # ---------- pass 1 (vertical), all 3 kernels per matmul ------
                # psum planes: [G | 2*M_sm | 2*M_df] applied down the rows;
                # one plain Copy extract per col-chunk (scales live in W).
                tTc = []
                for c in range(NCHUNK):
                    pv = psum.tile([128, 3 * N], F32, tag="cat")
                    pv3 = pv[:].rearrange("p (k w) -> p k w", w=N)
                    lhsT = [
                        xh[i][:, N * r + 128 * c : N * r + 128 * (c + 1)]
                        for r in range(NCHUNK)
                    ]
                    # one matmul per kernel-plane: a psum AP must stay inside
                    # a single 2KB psum bank
                    covered = 0
                    for r in range(NCHUNK):
                        lo, hi = R7[r]
                        first = r == 0
                        base = R7[r][0]
                        if not first and lo < covered:
                            for k in range(3):
                                nc.tensor.matmul(
                                    pv3[:, k, lo:covered], lhsT[r],
                                    w4[:, k, r, lo - base : covered - base],
                                    start=False, stop=False,
                                )
                            lo = covered
                        for k in range(3):
                            nc.tensor.matmul(
                                pv3[:, k, lo:hi], lhsT[r],
                                w4[:, k, r, lo - base : hi - base],
                                start=first, stop=(r == NCHUNK - 1),
                            )
                        covered = hi
                    st = tTpool.tile([128, 3 * N], F16, tag=f"tT_{c}")
                    if i == 0 and c % 2 == 0:
                        # during pipeline fill DVE is idle: split image 0's
                        # extracts between Act and DVE to start sooner
                        nc.vector.tensor_scalar(out=st[:], in0=pv[:],
                                                scalar1=1.0, op0=ALU.mult,
                                                scalar2=None)
                    else:
                        nc.scalar.activation(st[:], pv[:], AF.Copy)
                    tTc.append(st)
                # pass-2 stationary slices: tTc[c][:, k*512 + rows]
                tT = {
                    k: [tTc[c][:, kk * N : (kk + 1) * N] for c in range(NCHUNK)]
                    for kk, k in enumerate(("b", "x", "y"))
                }

                # ---------- pass 2 (horizontal) into cat psum [128, 3*512] ----
                # slice 0: b ; slice 1: 2*gx (then 2*(gx+gy)) ; slice 2: 2*gy
                # Eq holds the Abs-extract [S | A | Y] in padded quad layout
                # [128, 3, 4, 514]: S = s*b, A = 2s|gx|, Y = 2s|gy|
                Eq = post.tile([128, 3 * NCHUNK * W2], F16, tag="Eq")
                E4 = Eq[:].rearrange("p (k q w) -> p k q w", k=3, w=W2)
                # zero the S-plane column pads (cols 0 and 513 of each q)
                nc.gpsimd.memset(E4[:, 0, :, 0:1], 0.0)
                nc.gpsimd.memset(E4[:, 0, :, N + 1 : N + 2], 0.0)
                Pq = post.tile([128, NCHUNK * N], F16, tag="Pq")

                for rt in range(NCHUNK):
                    row0 = 128 * rt
                    p2 = psum.tile([128, 3 * N], F32, tag="cat")

                    def sl(key, c):
                        return tT[key][c][:, row0 : row0 + 128]

                    _banded_mm(nc, p2, [sl("b", c) for c in range(NCHUNK)],
                               w_sb["g"], stop=True, out_off=0)
                    _banded_mm(nc, p2, [sl("x", c) for c in range(NCHUNK)],
                               w_sb["mdf"], stop=True, out_off=N)
                    _banded_mm(nc, p2, [sl("y", c) for c in range(NCHUNK)],
                               w_sb["msm"], stop=True, out_off=2 * N)

                    # one Abs extract of all 3 slices; scale s gives
                    # S = s|b| = s*b, A = 2s|gx|, Y = 2s|gy|
                    nc.scalar.activation(
                        E4[:, :, rt, 1 : N + 1],
                        p2[:].rearrange("p (k w) -> p k w", w=N),
                        AF.Abs,
                        scale=s,
                    )
                    # u: gx-slice += 2*gy -> 2*(gx+gy); P = (s/sqrt2)*|2u|
                    #   mneg test: |u| < sqrt2|gx| <=> P < A
                    _accum_mm(nc, p2, [sl("y", c) for c in range(NCHUNK)],
                              w_sb["msm"], stop=True, out_off=N,
                              skip_group_check=True)
                    nc.scalar.activation(
                        Pq[:, rt * N : (rt + 1) * N],
                        p2[:, N : 2 * N],
                        AF.Abs,
                        scale=s / RT2,
                    )

                # ---------- Up/Dn shifted copies of the S plane via DMA ------
                # (issued per half so the select chain starts before the whole
                # image's pass-2 finishes)
                Upq = post.tile([128, NCHUNK * W2], F16, tag="Up")
                Dnq = post.tile([128, NCHUNK * W2], F16, tag="Dn")
                S3 = E4[:, 0]  # [128, 4, 514]
                U3 = Upq[:].rearrange("p (q w) -> p q w", w=W2)
                D3 = Dnq[:].rearrange("p (q w) -> p q w", w=W2)
                # boundary zeros (image edge rows): Up[127, 3] = 0; Dn[0, 0] = 0
                # (engine ops need quadrant-aligned start partitions; the
                # partition-127 sliver goes via DMA from the zero row)
                nc.sync.dma_start(
                    out=U3[127:128, 3:4, :],
                    in_=zrow[:].rearrange("p (q w) -> p q w", w=W2),
                )
                nc.gpsimd.memset(D3[0:1, 0:1, :], 0.0)

                A3 = E4[:, 1, :, 1 : N + 1]   # 4s|gx|
                Y3v = E4[:, 2, :, 1 : N + 1]  # 4s|gy|
                P3 = Pq[:].rearrange("p (q w) -> p q w", w=N)

                def qt(pool, tag, dt=F16):
                    t = pool.tile([128, NCHUNK * N], dt, tag=tag)
                    return t, t[:].rearrange("p (q w) -> p q w", w=N)

                Yloq, Ylo3 = qt(post1, "Ylo")  # Y/t1  (m0 = A >= Ylo)
                Yhiq, Yhi3 = qt(post1, "Yhi")  # Y*t1  (m2 = Yhi > A)
                m0q, m03 = qt(post1, "m0", U16)
                m2q, m23 = qt(post1, "m2", U16)
                mnq, mn3 = qt(post1, "mn", U16)
                selq, sel3 = qt(post, "sel")
                pmAq, pmA3 = qt(post, "pmA")
                pmVq, pmV3 = qt(post, "pmV")
                pmHq, pmH3 = qt(post, "pmH")
                Cq, C3 = qt(post1, "C")
                zq, z3 = qt(post1, "z")
                zrq, zr3 = qt(post1, "zr")
                oq = outp.tile([128, NCHUNK * N], F16, tag="o")
                o3 = oq[:].rearrange("p (q w) -> p q w", w=N)
                x3 = xh[i].rearrange("p (q w) -> p q w", w=N)

                TTv = nc.vector.tensor_tensor
                TTp = nc.gpsimd.tensor_tensor
                TSv = nc.vector.tensor_scalar

                # the select stage runs per image-half: shorter dependency
                # chains pipeline better across engines; the last image's
                # second half runs as quarters to shorten the drain tail
                if i == 0:
                    parts = [(0, 1), (1, 2), (2, 4)]
                else:
                    parts = [(0, 2), (2, 4)]
                for q0, q1 in parts:
                    hs = slice(q0, q1)
                    # Up/Dn: main shift + wrap rows
                    nc.sync.dma_start(out=U3[0:127, hs, :],
                                      in_=S3[1:128, hs, :])
                    nc.sync.dma_start(out=D3[1:128, hs, :],
                                      in_=S3[0:127, hs, :])
                    # Up[127, q] = S[0, q+1] (q < 3)
                    qh = min(q1, 3)
                    if qh > q0:
                        nc.sync.dma_start(out=U3[127:128, q0:qh, :],
                                          in_=S3[0:1, q0 + 1 : qh + 1, :])
                    # Dn[0, q] = S[127, q-1] (q > 0)
                    ql = max(q0, 1)
                    if q1 > ql:
                        nc.sync.dma_start(out=D3[0:1, ql:q1, :],
                                          in_=S3[127:128, ql - 1 : q1 - 1, :])

                    # scaled |gy| copies (DVE tensor_scalar, 4x mode)
                    TSv(out=Ylo3[:, hs], in0=Y3v[:, hs], scalar1=1.0 / T1,
                        op0=ALU.mult, scalar2=None)
                    TSv(out=Yhi3[:, hs], in0=Y3v[:, hs], scalar1=T1,
                        op0=ALU.mult, scalar2=None)
                    # masks (DVE tensor_tensor, 2x mode)
                    TTv(out=m03[:, hs], in0=A3[:, hs], in1=Ylo3[:, hs],
                        op=ALU.is_ge)
                    TTv(out=m23[:, hs], in0=Yhi3[:, hs], in1=A3[:, hs],
                        op=ALU.is_gt)
                    TTv(out=mn3[:, hs], in0=A3[:, hs], in1=P3[:, hs],
                        op=ALU.is_gt)
                    # pair maxes (DVE: Pool's GPSIMD has no max kernel)
                    TTv(out=sel3[:, hs], in0=D3[:, hs, 0:N],
                        in1=U3[:, hs, 2 : N + 2], op=ALU.max)  # main diag
                    TTv(out=pmA3[:, hs], in0=D3[:, hs, 2 : N + 2],
                        in1=U3[:, hs, 0:N], op=ALU.max)  # anti diag
                    TTv(out=pmV3[:, hs], in0=D3[:, hs, 1 : N + 1],
                        in1=U3[:, hs, 1 : N + 1], op=ALU.max)  # vertical
                    TTv(out=pmH3[:, hs], in0=S3[:, hs, 0:N],
                        in1=S3[:, hs, 2 : N + 2], op=ALU.max)  # horizontal
                    # selection cascade (DVE copy_predicated)
                    nc.vector.copy_predicated(sel3[:, hs], mn3[:, hs],
                                              pmA3[:, hs])
                    nc.vector.copy_predicated(sel3[:, hs], m23[:, hs],
                                              pmV3[:, hs])
                    nc.vector.copy_predicated(sel3[:, hs], m03[:, hs],
                                              pmH3[:, hs])
                    # z = zs*S - sel ; out = relu(z) * x
                    # Pool runs this chain (ts_mult/tt_sub/ts_max/tt_mult are
                    # the GPSIMD kernels that exist); last image on DVE for a
                    # short drain tail.
                    TSx = TSv if i == B_CORE - 1 else nc.gpsimd.tensor_scalar
                    TTc = TTv if i == B_CORE - 1 else TTp
                    TSx(out=C3[:, hs], in0=S3[:, hs, 1 : N + 1], scalar1=zs,
                        op0=ALU.mult, scalar2=None)
                    TTc(out=z3[:, hs], in0=C3[:, hs], in1=sel3[:, hs],
                        op=ALU.subtract)
                    TSx(out=zr3[:, hs], in0=z3[:, hs], scalar1=0.0,
                        op0=ALU.max, scalar2=None)
                    TTc(out=o3[:, hs], in0=zr3[:, hs], in1=x3[:, hs],
                        op=ALU.mult)
                    # store this part (fp16 -> fp32 cast)
                    nc.gpsimd.dma_start(
                        out=out_d[i * N + 128 * q0 : i * N + 128 * q1, :]
                        .rearrange("(r p) w -> p r w", p=128),
                        in_=o3[:, hs],
                    )

    nc.compile()
    return nc


# ---------------------------------------------------------------------------
# host side
# ---------------------------------------------------------------------------

def _make_band(weights, offsets, pad):
    M = np.zeros((N, N), dtype=np.float64)
    for w, o in zip(weights, offsets):
        idx = np.arange(N)
        src = idx + o
        if pad == "replicate":
            np.add.at(M, (np.clip(src, 0, N - 1), idx), w)
        else:
            ok = (src >= 0) & (src < N)
            np.add.at(M, (src[ok], idx[ok]), w)
    return M


def _host_weights(gauss_kernel):
    gk = np.asarray(gauss_kernel, dtype=np.float64)[0, 0]
    U, sv, Vt = np.linalg.svd(gk)
    assert sv[1] < 1e-5 * sv[0], "gauss kernel not rank-1 separable"
    wv = U[:, 0] * np.sqrt(sv[0])
    wh = Vt[0] * np.sqrt(sv[0])
    if wv.sum() < 0:
        wv, wh = -wv, -wh
    # symmetry of the gauss kernel => Bv == Bh (same 1-D factor both ways)
    assert np.allclose(wv, wh, atol=1e-12), "gauss kernel not symmetric"
    o5 = [-2, -1, 0, 1, 2]
    o3 = [-1, 0, 1]
    G = _make_band(wv, o5, "zero")
    Sm = _make_band([1, 2, 1], o3, "replicate")
    Df = _make_band([-1, 0, 1], o3, "replicate")
    full = [G, 2.0 * (G @ Sm), 2.0 * (G @ Df)]
    # compact band strips: per (kernel, row-chunk) only cols R7[r] are read
    strips = np.zeros((3, 4, 128, 136), dtype=np.float64)
    for k in range(3):
        for r in range(4):
            lo = max(0, 128 * r - 3)
            hi = min(512, 128 * r + 131)
            strips[k, r, :, : hi - lo] = full[k][128 * r : 128 * (r + 1), lo:hi]
    return {"wcat": np.ascontiguousarray(
        strips.reshape(3 * 4 * 128, 136), dtype=np.float16)}



_NC_CACHE = {}
LAST_RESULT = None


def kernel(reconst, gauss_kernel, nms_kernel):
    nk = np.asarray(nms_kernel, dtype=np.float64)
    cen = float(nk[0, 0, 1, 1])
    v = float(nk[0, 0, 1, 2])
    # verify nms kernel structure: center + single tap v per direction
    pos = [(1, 2), (2, 2), (2, 1), (2, 0), (1, 0), (0, 0), (0, 1), (0, 2)]
    for d, (r, c) in enumerate(pos):
        k = nk[d, 0].copy()
        assert abs(k[1, 1] - cen) < 1e-6 and abs(k[r, c] - v) < 1e-6
        k[1, 1] = 0.0
        k[r, c] = 0.0
        assert np.abs(k).max() < 1e-7
    assert v < 0

    key = (round(cen, 9), round(v, 9))
    if key not in _NC_CACHE:
        _NC_CACHE[key] = build_nc(cen, v)
    nc = _NC_CACHE[key]

    w = _host_weights(gauss_kernel)
    x = np.asarray(reconst, dtype=np.float32).reshape(B_TOTAL, N, N)
    in_maps = []
    for core in range(N_CORES):
        m = {"x": np.ascontiguousarray(
            x[core * B_CORE : (core + 1) * B_CORE].reshape(B_CORE * N, N)
        )}
        m.update(w)
        in_maps.append(m)

    res = run_bass_kernel_spmd(nc, in_maps, core_ids=list(range(N_CORES)))
    global LAST_RESULT
    LAST_RESULT = res
    out = np.concatenate(
        [r["out"].reshape(B_CORE, 1, N, N) for r in res.results], axis=0
    )
    return out.astype(np.float32)


# revision 76
# speedup vs baseline: 1.0356x; 1.0050x over previous
"""Trainium2 Bass kernel for nn_Densenet_with_skip (gauss blur -> sobel ->
angle-binned 8-direction NMS -> gate).

Reformulation (same math as the validated baseline):
  b  = gauss5x5(x)                      (zero pad; separable, rank-1)
  gx/gy via composed 7-tap band matmuls (replicate pad on b)
  bin: m0 = (t1*|gx| >= |gy|)  -> horizontal pair
       m2 = (t1*|gy| >  |gx|)  -> vertical pair
       else diag: gx*gy<0 (<=> |gx+gy| < sqrt2*|gx|) -> anti-diag pair
  out = x * relu(cen*b + v*max(pair))

v2 engine-balance rewrite (vs the STT-heavy baseline):
  - 3 deduped weight matrices (gauss is symmetric: Bv==Bh; Sv==Sh; Dv==Dh),
    shared between both matmul passes; scales folded into the pass-1/pass-2
    PSUM->SBUF extract `scale` so every elementwise compare is a plain
    fp16 tensor_tensor (2x DVE mode) or tensor_scalar (4x DVE mode).
  - pass-2 accumulates [b | gx | gy] into ONE 3-bank PSUM tile per row-tile
    so a single Abs-activation extracts S/A/Y together (b >= 0 so Abs is a
    no-op on the b slice); |gx+gy| rides the same psum via 4 extra matmuls.
  - engine split: masks/pair-maxes/selection (copy_predicated, u16 masks)
    on DVE; the zs*S-sel / relu / gate chain on Pool (only add/sub/mult/
    ts_mult/ts_max exist as GPSIMD kernels; max/is_ge/STT do not compile);
    PSUM->SBUF extracts on Act; last image's chain on DVE for a short tail.
  - per-image-half select stage for cross-engine pipelining; batched DMAs
    (x load / out store); Up/Dn row-shifted copies of S on the DMA engines.
"""

import sys

import numpy as np

sys.path.insert(0, "/opt/trn_rl_repo")

import concourse.bacc as bacc
import concourse.mybir as mybir
from concourse import tile
from concourse.bass_utils import run_bass_kernel_spmd

N = 512
B_TOTAL = 32
N_CORES = 8
B_CORE = B_TOTAL // N_CORES  # 4 images per core
NCHUNK = N // 128  # 4

F16 = mybir.dt.float16
F32 = mybir.dt.float32
U16 = mybir.dt.uint16

T1 = float(np.tan(np.pi / 8))  # tan(22.5 deg)
RT2 = float(np.sqrt(2.0))

ALU = mybir.AluOpType
AF = mybir.ActivationFunctionType


def _band_ranges(halo):
    out = []
    for r in range(NCHUNK):
        lo = max(0, 128 * r - halo)
        hi = min(N, 128 * r + 128 + halo)
        out.append((lo, hi))
    return out


R7 = _band_ranges(3)
WSTRIP = 136  # compact W strip width (>= max band range width 134)


def _banded_mm(nc, psum_ap, lhsT_sl, w_chunks, start_fresh=True, stop=True,
               out_off=0):
    """Accumulate sum_r lhsT_r.T @ W_r[:, band] into psum[:, out_off + band]
    with PSUM fresh/covered split handling (each matmul touches either
    all-fresh or all-covered columns)."""
    covered = 0
    n = NCHUNK
    for r in range(n):
        lo, hi = R7[r]
        first = r == 0
        last = r == n - 1
        base = R7[r][0]
        if not first and lo < covered:
            nc.tensor.matmul(
                psum_ap[:, out_off + lo : out_off + covered],
                lhsT_sl[r],
                w_chunks[:, r, lo - base : covered - base],
                start=False,
                stop=False,
            )
            lo = covered
        nc.tensor.matmul(
            psum_ap[:, out_off + lo : out_off + hi],
            lhsT_sl[r],
            w_chunks[:, r, lo - base : hi - base],
            start=first and start_fresh,
            stop=last and stop,
        )
        covered = hi


def _accum_mm(nc, psum_ap, lhsT_sl, w_chunks, stop=True, out_off=0,
              skip_group_check=False):
    """Accumulate onto an already-covered psum range (no splits needed)."""
    for r in range(NCHUNK):
        lo, hi = R7[r]
        base = R7[r][0]
        nc.tensor.matmul(
            psum_ap[:, out_off + lo : out_off + hi],
            lhsT_sl[r],
            w_chunks[:, r, lo - base : hi - base],
            start=False,
            stop=stop and (r == NCHUNK - 1),
            skip_group_check=skip_group_check,
        )


def build_nc(cen, v):
    s = -v  # S = s*b ; z = (cen/s)*S - sel
    zs = cen / s

    nc = bacc.Bacc("TRN2", target_bir_lowering=False, debug=False)

    x_d = nc.dram_tensor("x", [B_CORE * N, N], F32, kind="ExternalInput").ap()
    # three deduped band matrices as one cat tensor [3, 512, 512] fp16:
    # plane 0: G (gauss 5-tap), plane 1: 2*M_sm (Bv@Sm), plane 2: 2*M_df
    wcat_d = nc.dram_tensor(
        "wcat", [3 * NCHUNK * 128, WSTRIP], F16, kind="ExternalInput"
    ).ap()
    out_d = nc.dram_tensor("out", [B_CORE * N, N], F32, kind="ExternalOutput").ap()

    W2 = N + 2  # padded row width for col-shifted access

    with tile.TileContext(nc) as tc:
        with (
            tc.tile_pool(name="wpool", bufs=1) as wpool,
            tc.tile_pool(name="xpool", bufs=1) as xpool,
            tc.tile_pool(name="tT", bufs=2) as tTpool,
            tc.tile_pool(name="post", bufs=2) as post,
            tc.tile_pool(name="post1", bufs=1) as post1,
            tc.tile_pool(name="outp", bufs=2) as outp,
            tc.tile_pool(name="psum", bufs=2, space="PSUM") as psum,
        ):
            zrow = wpool.tile([1, W2], F16, tag="zrow")
            nc.vector.memset(zrow[:], 0.0)

            # --- load x as fp16 (DMA cast); quad layout [128, 4*512]:
            # image 0 alone (so compute starts early), images 1-3 batched
            x0t = xpool.tile([128, NCHUNK * N], F16, tag="xh_0")
            nc.gpsimd.dma_start(
                out=x0t[:].rearrange("p (r w) -> p r w", w=N),
                in_=x_d[0:N, :].rearrange("(r p) w -> p r w", p=128),
            )
            xh = [x0t[:], None, None, None]

            # --- weight cat [128, 3, 4, 512]; one DMA per row-chunk so the
            # first pass-1 matmuls can start early
            wt = wpool.tile([128, 3 * NCHUNK * WSTRIP], F16, tag="wcat")
            w4 = wt[:].rearrange("p (k r w) -> p k r w", k=3, w=WSTRIP)
            wsrc = wcat_d.rearrange("(k r p) w -> p r k w", k=3, p=128)
            for r in range(NCHUNK):
                nc.sync.dma_start(out=w4[:, :, r, :], in_=wsrc[:, r])
            w_sb = {"g": w4[:, 0], "msm": w4[:, 1], "mdf": w4[:, 2]}

            # images 1-3 in one DMA (overlaps image-0 compute)
            x13 = xpool.tile([128, 3 * NCHUNK * N], F16, tag="xh_13")
            nc.gpsimd.dma_start(
                out=x13[:].rearrange("p (i r w) -> p i r w", i=3, w=N),
                in_=x_d[N:, :].rearrange("(i r p) w -> p i r w", p=128, i=3),
            )
            for i in range(1, B_CORE):
                xh[i] = x13[:, (i - 1) * NCHUNK * N : i * NCHUNK * N]

            for i in range(B_CORE):
                # ---------- pass 1 (vertical), all 3 kernels per matmul ------
                # psum planes: [G | 2*M_sm | 2*M_df] applied down the rows;
                # one plain Copy extract per col-chunk (scales live in W).
                tTc = []
                for c in range(NCHUNK):
                    pv = psum.tile([128, 3 * N], F32, tag="cat")
                    pv3 = pv[:].rearrange("p (k w) -> p k w", w=N)
                    lhsT = [
                        xh[i][:, N * r + 128 * c : N * r + 128 * (c + 1)]
                        for r in range(NCHUNK)
                    ]
                    # one matmul per kernel-plane: a psum AP must stay inside
                    # a single 2KB psum bank
                    covered = 0
                    for r in range(NCHUNK):
                        lo, hi = R7[r]
                        first = r == 0
                        base = R7[r][0]
                        if not first and lo < covered:
                            for k in range(3):
                                nc.tensor.matmul(
                                    pv3[:, k, lo:covered], lhsT[r],
                                    w4[:, k, r, lo - base : covered - base],
                                    start=False, stop=False,
                                )
                            lo = covered
                        for k in range(3):
                            nc.tensor.matmul(
                                pv3[:, k, lo:hi], lhsT[r],
                                w4[:, k, r, lo - base : hi - base],
                                start=first, stop=(r == NCHUNK - 1),
                            )
                        covered = hi
                    st = tTpool.tile([128, 3 * N], F16, tag=f"tT_{c}")
                    if i == 0 and c % 2 == 0:
                        # during pipeline fill DVE is idle: split image 0's
                        # extracts between Act and DVE to start sooner
                        nc.vector.tensor_scalar(out=st[:], in0=pv[:],
                                                scalar1=1.0, op0=ALU.mult,
                                                scalar2=None)
                    else:
                        nc.scalar.activation(st[:], pv[:], AF.Copy)
                    tTc.append(st)
                # pass-2 stationary slices: tTc[c][:, k*512 + rows]
                tT = {
                    k: [tTc[c][:, kk * N : (kk + 1) * N] for c in range(NCHUNK)]
                    for kk, k in enumerate(("b", "x", "y"))
                }

                # ---------- pass 2 (horizontal) into cat psum [128, 3*512] ----
                # slice 0: b ; slice 1: 2*gx (then 2*(gx+gy)) ; slice 2: 2*gy
                # Eq holds the Abs-extract [S | A | Y] in padded quad layout
                # [128, 3, 4, 514]: S = s*b, A = 2s|gx|, Y = 2s|gy|
                Eq = post.tile([128, 3 * NCHUNK * W2], F16, tag="Eq")
                E4 = Eq[:].rearrange("p (k q w) -> p k q w", k=3, w=W2)
                # zero the S-plane column pads (cols 0 and 513 of each q)
                nc.gpsimd.memset(E4[:, 0, :, 0:1], 0.0)
                nc.gpsimd.memset(E4[:, 0, :, N + 1 : N + 2], 0.0)
                Pq = post.tile([128, NCHUNK * N], F16, tag="Pq")

                for rt in range(NCHUNK):
                    row0 = 128 * rt
                    p2 = psum.tile([128, 3 * N], F32, tag="cat")

                    def sl(key, c):
                        return tT[key][c][:, row0 : row0 + 128]

                    _banded_mm(nc, p2, [sl("b", c) for c in range(NCHUNK)],
                               w_sb["g"], stop=True, out_off=0)
                    _banded_mm(nc, p2, [sl("x", c) for c in range(NCHUNK)],
                               w_sb["mdf"], stop=True, out_off=N)
                    _banded_mm(nc, p2, [sl("y", c) for c in range(NCHUNK)],
                               w_sb["msm"], stop=True, out_off=2 * N)

                    # one Abs extract of all 3 slices; scale s gives
                    # S = s|b| = s*b, A = 2s|gx|, Y = 2s|gy|
                    nc.scalar.activation(
                        E4[:, :, rt, 1 : N + 1],
                        p2[:].rearrange("p (k w) -> p k w", w=N),
                        AF.Abs,
                        scale=s,
                    )
                    # u: gx-slice += 2*gy -> 2*(gx+gy); P = (s/sqrt2)*|2u|
                    #   mneg test: |u| < sqrt2|gx| <=> P < A
                    _accum_mm(nc, p2, [sl("y", c) for c in range(NCHUNK)],
                              w_sb["msm"], stop=True, out_off=N,
                              skip_group_check=True)
                    nc.scalar.activation(
                        Pq[:, rt * N : (rt + 1) * N],
                        p2[:, N : 2 * N],
                        AF.Abs,
                        scale=s / RT2,
                    )

                # ---------- Up/Dn shifted copies of the S plane via DMA ------
                # (issued per half so the select chain starts before the whole
                # image's pass-2 finishes)
                Upq = post.tile([128, NCHUNK * W2], F16, tag="Up")
                Dnq = post.tile([128, NCHUNK * W2], F16, tag="Dn")
                S3 = E4[:, 0]  # [128, 4, 514]
                U3 = Upq[:].rearrange("p (q w) -> p q w", w=W2)
                D3 = Dnq[:].rearrange("p (q w) -> p q w", w=W2)
                # boundary zeros (image edge rows): Up[127, 3] = 0; Dn[0, 0] = 0
                # (engine ops need quadrant-aligned start partitions; the
                # partition-127 sliver goes via DMA from the zero row)
                nc.sync.dma_start(
                    out=U3[127:128, 3:4, :],
                    in_=zrow[:].rearrange("p (q w) -> p q w", w=W2),
                )
                nc.gpsimd.memset(D3[0:1, 0:1, :], 0.0)

                A3 = E4[:, 1, :, 1 : N + 1]   # 4s|gx|
                Y3v = E4[:, 2, :, 1 : N + 1]  # 4s|gy|
                P3 = Pq[:].rearrange("p (q w) -> p q w", w=N)

                def qt(pool, tag, dt=F16):
                    t = pool.tile([128, NCHUNK * N], dt, tag=tag)
                    return t, t[:].rearrange("p (q w) -> p q w", w=N)

                Yloq, Ylo3 = qt(post1, "Ylo")  # Y/t1  (m0 = A >= Ylo)
                Yhiq, Yhi3 = qt(post1, "Yhi")  # Y*t1  (m2 = Yhi > A)
                m0q, m03 = qt(post1, "m0", U16)
                m2q, m23 = qt(post1, "m2", U16)
                mnq, mn3 = qt(post1, "mn", U16)
                selq, sel3 = qt(post, "sel")
                pmAq, pmA3 = qt(post, "pmA")
                pmVq, pmV3 = qt(post, "pmV")
                pmHq, pmH3 = qt(post, "pmH")
                Cq, C3 = qt(post1, "C")
                zq, z3 = qt(post1, "z")
                zrq, zr3 = qt(post1, "zr")
                oq = outp.tile([128, NCHUNK * N], F16, tag="o")
                o3 = oq[:].rearrange("p (q w) -> p q w", w=N)
                x3 = xh[i].rearrange("p (q w) -> p q w", w=N)

                TTv = nc.vector.tensor_tensor
                TTp = nc.gpsimd.tensor_tensor
                TSv = nc.vector.tensor_scalar

                # the select stage runs per image-half: shorter dependency
                # chains pipeline better across engines; the last image's
                # second half runs as quarters to shorten the drain tail
                if i == 0:
                    parts = [(0, 1), (1, 2), (2, 4)]
                else:
                    parts = [(0, 2), (2, 4)]
                for q0, q1 in parts:
                    hs = slice(q0, q1)
                    # Up/Dn: main shift + wrap rows
                    nc.sync.dma_start(out=U3[0:127, hs, :],
                                      in_=S3[1:128, hs, :])
                    nc.sync.dma_start(out=D3[1:128, hs, :],
                                      in_=S3[0:127, hs, :])
                    # Up[127, q] = S[0, q+1] (q < 3)
                    qh = min(q1, 3)
                    if qh > q0:
                        nc.sync.dma_start(out=U3[127:128, q0:qh, :],
                                          in_=S3[0:1, q0 + 1 : qh + 1, :])
                    # Dn[0, q] = S[127, q-1] (q > 0)
                    ql = max(q0, 1)
                    if q1 > ql:
                        nc.sync.dma_start(out=D3[0:1, ql:q1, :],
                                          in_=S3[127:128, ql - 1 : q1 - 1, :])

                    # scaled |gy| copies (DVE tensor_scalar, 4x mode)
                    TSv(out=Ylo3[:, hs], in0=Y3v[:, hs], scalar1=1.0 / T1,
                        op0=ALU.mult, scalar2=None)
                    TSv(out=Yhi3[:, hs], in0=Y3v[:, hs], scalar1=T1,
                        op0=ALU.mult, scalar2=None)
                    # masks (DVE tensor_tensor, 2x mode)
                    TTv(out=m03[:, hs], in0=A3[:, hs], in1=Ylo3[:, hs],
                        op=ALU.is_ge)
                    TTv(out=m23[:, hs], in0=Yhi3[:, hs], in1=A3[:, hs],
                        op=ALU.is_gt)
                    TTv(out=mn3[:, hs], in0=A3[:, hs], in1=P3[:, hs],
                        op=ALU.is_gt)
                    # pair maxes (DVE: Pool's GPSIMD has no max kernel)
                    TTv(out=sel3[:, hs], in0=D3[:, hs, 0:N],
                        in1=U3[:, hs, 2 : N + 2], op=ALU.max)  # main diag
                    TTv(out=pmA3[:, hs], in0=D3[:, hs, 2 : N + 2],
                        in1=U3[:, hs, 0:N], op=ALU.max)  # anti diag
                    TTv(out=pmV3[:, hs], in0=D3[:, hs, 1 : N + 1],
                        in1=U3[:, hs, 1 : N + 1], op=ALU.max)  # vertical
                    TTv(out=pmH3[:, hs], in0=S3[:, hs, 0:N],
                        in1=S3[:, hs, 2 : N + 2], op=ALU.max)  # horizontal
                    # selection cascade (DVE copy_predicated)
                    nc.vector.copy_predicated(sel3[:, hs], mn3[:, hs],
                                              pmA3[:, hs])
                    nc.vector.copy_predicated(sel3[:, hs], m23[:, hs],
                                              pmV3[:, hs])
                    nc.vector.copy_predicated(sel3[:, hs], m03[:, hs],
                                              pmH3[:, hs])
                    # z = zs*S - sel ; out = relu(z) * x
                    # Pool runs this chain (ts_mult/tt_sub/ts_max/tt_mult are
                    # the GPSIMD kernels that exist); last image on DVE for a
                    # short drain tail.
                    TSx = TSv if i == B_CORE - 1 else nc.gpsimd.tensor_scalar
                    TTc = TTv if i == B_CORE - 1 else TTp
                    # C is off the select critical path: keep it on Pool even
                    # for the last image so it overlaps the cp cascade
                    nc.gpsimd.tensor_scalar(
                        out=C3[:, hs], in0=S3[:, hs, 1 : N + 1], scalar1=zs,
                        op0=ALU.mult, scalar2=None)
                    TTc(out=z3[:, hs], in0=C3[:, hs], in1=sel3[:, hs],
                        op=ALU.subtract)
                    TSx(out=zr3[:, hs], in0=z3[:, hs], scalar1=0.0,
                        op0=ALU.max, scalar2=None)
                    TTc(out=o3[:, hs], in0=zr3[:, hs], in1=x3[:, hs],
                        op=ALU.mult)
                    # store this part (fp16 -> fp32 cast)
                    nc.gpsimd.dma_start(
                        out=out_d[i * N + 128 * q0 : i * N + 128 * q1, :]
                        .rearrange("(r p) w -> p r w", p=128),
                        in_=o3[:, hs],
                    )

    nc.compile()
    return nc


# ---------------------------------------------------------------------------
# host side
# ---------------------------------------------------------------------------

def _make_band(weights, offsets, pad):
    M = np.zeros((N, N), dtype=np.float64)
    for w, o in zip(weights, offsets):
        idx = np.arange(N)
        src = idx + o
        if pad == "replicate":
            np.add.at(M, (np.clip(src, 0, N - 1), idx), w)
        else:
            ok = (src >= 0) & (src < N)
            np.add.at(M, (src[ok], idx[ok]), w)
    return M


def _host_weights(gauss_kernel):
    gk = np.asarray(gauss_kernel, dtype=np.float64)[0, 0]
    U, sv, Vt = np.linalg.svd(gk)
    assert sv[1] < 1e-5 * sv[0], "gauss kernel not rank-1 separable"
    wv = U[:, 0] * np.sqrt(sv[0])
    wh = Vt[0] * np.sqrt(sv[0])
    if wv.sum() < 0:
        wv, wh = -wv, -wh
    # symmetry of the gauss kernel => Bv == Bh (same 1-D factor both ways)
    assert np.allclose(wv, wh, atol=1e-12), "gauss kernel not symmetric"
    o5 = [-2, -1, 0, 1, 2]
    o3 = [-1, 0, 1]
    G = _make_band(wv, o5, "zero")
    Sm = _make_band([1, 2, 1], o3, "replicate")
    Df = _make_band([-1, 0, 1], o3, "replicate")
    full = [G, 2.0 * (G @ Sm), 2.0 * (G @ Df)]
    # compact band strips: per (kernel, row-chunk) only cols R7[r] are read
    strips = np.zeros((3, 4, 128, 136), dtype=np.float64)
    for k in range(3):
        for r in range(4):
            lo = max(0, 128 * r - 3)
            hi = min(512, 128 * r + 131)
            strips[k, r, :, : hi - lo] = full[k][128 * r : 128 * (r + 1), lo:hi]
    return {"wcat": np.ascontiguousarray(
        strips.reshape(3 * 4 * 128, 136), dtype=np.float16)}



_NC_CACHE = {}
LAST_RESULT = None


def kernel(reconst, gauss_kernel, nms_kernel):
    nk = np.asarray(nms_kernel, dtype=np.float64)
    cen = float(nk[0, 0, 1, 1])
    v = float(nk[0, 0, 1, 2])
    # verify nms kernel structure: center + single tap v per direction
    pos = [(1, 2), (2, 2), (2, 1), (2, 0), (1, 0), (0, 0), (0, 1), (0, 2)]
    for d, (r, c) in enumerate(pos):
        k = nk[d, 0].copy()
        assert abs(k[1, 1] - cen) < 1e-6 and abs(k[r, c] - v) < 1e-6
        k[1, 1] = 0.0
        k[r, c] = 0.0
        assert np.abs(k).max() < 1e-7
    assert v < 0

    key = (round(cen, 9), round(v, 9))
    if key not in _NC_CACHE:
        _NC_CACHE[key] = build_nc(cen, v)
    nc = _NC_CACHE[key]

    w = _host_weights(gauss_kernel)
    x = np.asarray(reconst, dtype=np.float32).reshape(B_TOTAL, N, N)
    in_maps = []
    for core in range(N_CORES):
        m = {"x": np.ascontiguousarray(
            x[core * B_CORE : (core + 1) * B_CORE].reshape(B_CORE * N, N)
        )}
        m.update(w)
        in_maps.append(m)

    res = run_bass_kernel_spmd(nc, in_maps, core_ids=list(range(N_CORES)))
    global LAST_RESULT
    LAST_RESULT = res
    out = np.concatenate(
        [r["out"].reshape(B_CORE, 1, N, N) for r in res.results], axis=0
    )
    return out.astype(np.float32)


# revision 78
# speedup vs baseline: 1.0448x; 1.0089x over previous
"""Trainium2 Bass kernel for nn_Densenet_with_skip (gauss blur -> sobel ->
angle-binned 8-direction NMS -> gate).

Reformulation (same math as the validated baseline):
  b  = gauss5x5(x)                      (zero pad; separable, rank-1)
  gx/gy via composed 7-tap band matmuls (replicate pad on b)
  bin: m0 = (t1*|gx| >= |gy|)  -> horizontal pair
       m2 = (t1*|gy| >  |gx|)  -> vertical pair
       else diag: gx*gy<0 (<=> |gx+gy| < sqrt2*|gx|) -> anti-diag pair
  out = x * relu(cen*b + v*max(pair))

v2 engine-balance rewrite (vs the STT-heavy baseline):
  - 3 deduped weight matrices (gauss is symmetric: Bv==Bh; Sv==Sh; Dv==Dh),
    shared between both matmul passes; scales folded into the pass-1/pass-2
    PSUM->SBUF extract `scale` so every elementwise compare is a plain
    fp16 tensor_tensor (2x DVE mode) or tensor_scalar (4x DVE mode).
  - pass-2 accumulates [b | gx | gy] into ONE 3-bank PSUM tile per row-tile
    so a single Abs-activation extracts S/A/Y together (b >= 0 so Abs is a
    no-op on the b slice); |gx+gy| rides the same psum via 4 extra matmuls.
  - engine split: masks/pair-maxes/selection (copy_predicated, u16 masks)
    on DVE; the zs*S-sel / relu / gate chain on Pool (only add/sub/mult/
    ts_mult/ts_max exist as GPSIMD kernels; max/is_ge/STT do not compile);
    PSUM->SBUF extracts on Act; last image's chain on DVE for a short tail.
  - per-image-half select stage for cross-engine pipelining; batched DMAs
    (x load / out store); Up/Dn row-shifted copies of S on the DMA engines.
"""

import sys

import numpy as np

sys.path.insert(0, "/opt/trn_rl_repo")

import concourse.bacc as bacc
import concourse.mybir as mybir
from concourse import tile
from concourse.bass_utils import run_bass_kernel_spmd

N = 512
B_TOTAL = 32
N_CORES = 8
B_CORE = B_TOTAL // N_CORES  # 4 images per core
NCHUNK = N // 128  # 4

F16 = mybir.dt.float16
F32 = mybir.dt.float32
U16 = mybir.dt.uint16

T1 = float(np.tan(np.pi / 8))  # tan(22.5 deg)
RT2 = float(np.sqrt(2.0))

ALU = mybir.AluOpType
AF = mybir.ActivationFunctionType


def _band_ranges(halo):
    out = []
    for r in range(NCHUNK):
        lo = max(0, 128 * r - halo)
        hi = min(N, 128 * r + 128 + halo)
        out.append((lo, hi))
    return out


R7 = _band_ranges(3)
WSTRIP = 136  # compact W strip width (>= max band range width 134)


def _banded_mm(nc, psum_ap, lhsT_sl, w_chunks, start_fresh=True, stop=True,
               out_off=0):
    """Accumulate sum_r lhsT_r.T @ W_r[:, band] into psum[:, out_off + band]
    with PSUM fresh/covered split handling (each matmul touches either
    all-fresh or all-covered columns)."""
    covered = 0
    n = NCHUNK
    for r in range(n):
        lo, hi = R7[r]
        first = r == 0
        last = r == n - 1
        base = R7[r][0]
        if not first and lo < covered:
            nc.tensor.matmul(
                psum_ap[:, out_off + lo : out_off + covered],
                lhsT_sl[r],
                w_chunks[:, r, lo - base : covered - base],
                start=False,
                stop=False,
            )
            lo = covered
        nc.tensor.matmul(
            psum_ap[:, out_off + lo : out_off + hi],
            lhsT_sl[r],
            w_chunks[:, r, lo - base : hi - base],
            start=first and start_fresh,
            stop=last and stop,
        )
        covered = hi


def _accum_mm(nc, psum_ap, lhsT_sl, w_chunks, stop=True, out_off=0,
              skip_group_check=False):
    """Accumulate onto an already-covered psum range (no splits needed)."""
    for r in range(NCHUNK):
        lo, hi = R7[r]
        base = R7[r][0]
        nc.tensor.matmul(
            psum_ap[:, out_off + lo : out_off + hi],
            lhsT_sl[r],
            w_chunks[:, r, lo - base : hi - base],
            start=False,
            stop=stop and (r == NCHUNK - 1),
            skip_group_check=skip_group_check,
        )


def build_nc(cen, v):
    s = -v  # S = s*b ; z = (cen/s)*S - sel
    zs = cen / s

    nc = bacc.Bacc("TRN2", target_bir_lowering=False, debug=False)

    x_d = nc.dram_tensor("x", [B_CORE * N, N], F32, kind="ExternalInput").ap()
    # three deduped band matrices as one cat tensor [3, 512, 512] fp16:
    # plane 0: G (gauss 5-tap), plane 1: 2*M_sm (Bv@Sm), plane 2: 2*M_df
    wcat_d = nc.dram_tensor(
        "wcat", [4 * NCHUNK * 128, WSTRIP], F16, kind="ExternalInput"
    ).ap()
    out_d = nc.dram_tensor("out", [B_CORE * N, N], F32, kind="ExternalOutput").ap()

    W2 = N + 2  # padded row width for col-shifted access

    with tile.TileContext(nc) as tc:
        with (
            tc.tile_pool(name="wpool", bufs=1) as wpool,
            tc.tile_pool(name="xpool", bufs=1) as xpool,
            tc.tile_pool(name="tT", bufs=2) as tTpool,
            tc.tile_pool(name="post", bufs=2) as post,
            tc.tile_pool(name="post1", bufs=1) as post1,
            tc.tile_pool(name="outp", bufs=2) as outp,
            tc.tile_pool(name="psum", bufs=2, space="PSUM") as psum,
        ):
            zrow = wpool.tile([1, W2], F16, tag="zrow")
            nc.vector.memset(zrow[:], 0.0)

            # --- load x as fp16 (DMA cast); quad layout [128, 4*512]:
            # image 0 alone (so compute starts early), images 1-3 batched
            x0t = xpool.tile([128, NCHUNK * N], F16, tag="xh_0")
            nc.gpsimd.dma_start(
                out=x0t[:].rearrange("p (r w) -> p r w", w=N),
                in_=x_d[0:N, :].rearrange("(r p) w -> p r w", p=128),
            )
            xh = [x0t[:], None, None, None]

            # --- weight cat [128, 3, 4, 512]; one DMA per row-chunk so the
            # first pass-1 matmuls can start early
            wt = wpool.tile([128, 4 * NCHUNK * WSTRIP], F16, tag="wcat")
            w4 = wt[:].rearrange("p (k r w) -> p k r w", k=4, w=WSTRIP)
            wsrc = wcat_d.rearrange("(k r p) w -> p r k w", k=4, p=128)
            for r in range(NCHUNK):
                nc.sync.dma_start(out=w4[:, :, r, :], in_=wsrc[:, r])
            # plane 3 = t1*2*M_sm: the u-sweep needs gy at the SAME scale as
            # gx even though t_y carries an extra 1/t1 for the m0 threshold
            w_sb = {"g": w4[:, 0], "msm": w4[:, 1], "mdf": w4[:, 2],
                    "msmt": w4[:, 3]}

            # images 1-3 in one DMA (overlaps image-0 compute)
            x13 = xpool.tile([128, 3 * NCHUNK * N], F16, tag="xh_13")
            nc.gpsimd.dma_start(
                out=x13[:].rearrange("p (i r w) -> p i r w", i=3, w=N),
                in_=x_d[N:, :].rearrange("(i r p) w -> p i r w", p=128, i=3),
            )
            for i in range(1, B_CORE):
                xh[i] = x13[:, (i - 1) * NCHUNK * N : i * NCHUNK * N]

            for i in range(B_CORE):
                # ---------- pass 1 (vertical), all 3 kernels per matmul ------
                # psum planes: [G | 2*M_sm | 2*M_df] applied down the rows;
                # one plain Copy extract per col-chunk (scales live in W).
                tTc = []
                for c in range(NCHUNK):
                    pv = psum.tile([128, 3 * N], F32, tag="cat")
                    pv3 = pv[:].rearrange("p (k w) -> p k w", w=N)
                    lhsT = [
                        xh[i][:, N * r + 128 * c : N * r + 128 * (c + 1)]
                        for r in range(NCHUNK)
                    ]
                    # one matmul per kernel-plane: a psum AP must stay inside
                    # a single 2KB psum bank
                    covered = 0
                    for r in range(NCHUNK):
                        lo, hi = R7[r]
                        first = r == 0
                        base = R7[r][0]
                        if not first and lo < covered:
                            for k in range(3):
                                nc.tensor.matmul(
                                    pv3[:, k, lo:covered], lhsT[r],
                                    w4[:, k, r, lo - base : covered - base],
                                    start=False, stop=False,
                                )
                            lo = covered
                        for k in range(3):
                            nc.tensor.matmul(
                                pv3[:, k, lo:hi], lhsT[r],
                                w4[:, k, r, lo - base : hi - base],
                                start=first, stop=(r == NCHUNK - 1),
                            )
                        covered = hi
                    st = tTpool.tile([128, 3 * N], F16, tag=f"tT_{c}")
                    if i == 0 and c % 2 == 0:
                        # during pipeline fill DVE is idle: split image 0's
                        # extracts between Act and DVE to start sooner
                        nc.vector.tensor_scalar(out=st[:], in0=pv[:],
                                                scalar1=1.0, op0=ALU.mult,
                                                scalar2=None)
                    else:
                        nc.scalar.activation(st[:], pv[:], AF.Copy)
                    tTc.append(st)
                # pass-2 stationary slices: tTc[c][:, k*512 + rows]
                tT = {
                    k: [tTc[c][:, kk * N : (kk + 1) * N] for c in range(NCHUNK)]
                    for kk, k in enumerate(("b", "x", "y"))
                }

                # ---------- pass 2 (horizontal) into cat psum [128, 3*512] ----
                # slice 0: b ; slice 1: 2*gx (then 2*(gx+gy)) ; slice 2: 2*gy
                # Eq holds the Abs-extract [S | A | Y] in padded quad layout
                # [128, 3, 4, 514]: S = s*b, A = 2s|gx|, Y = 2s|gy|
                Eq = post.tile([128, 3 * NCHUNK * W2], F16, tag="Eq")
                E4 = Eq[:].rearrange("p (k q w) -> p k q w", k=3, w=W2)
                # zero the S-plane column pads (cols 0 and 513 of each q)
                nc.gpsimd.memset(E4[:, 0, :, 0:1], 0.0)
                nc.gpsimd.memset(E4[:, 0, :, N + 1 : N + 2], 0.0)
                Pq = post.tile([128, NCHUNK * N], F16, tag="Pq")

                for rt in range(NCHUNK):
                    row0 = 128 * rt
                    p2 = psum.tile([128, 3 * N], F32, tag="cat")

                    def sl(key, c):
                        return tT[key][c][:, row0 : row0 + 128]

                    _banded_mm(nc, p2, [sl("b", c) for c in range(NCHUNK)],
                               w_sb["g"], stop=True, out_off=0)
                    _banded_mm(nc, p2, [sl("x", c) for c in range(NCHUNK)],
                               w_sb["mdf"], stop=True, out_off=N)
                    _banded_mm(nc, p2, [sl("y", c) for c in range(NCHUNK)],
                               w_sb["msmt"], stop=True, out_off=2 * N)

                    # one Abs extract of all 3 slices; scale s gives
                    # S = s|b| = s*b, A = 2s|gx|, Y = 2s|gy|
                    nc.scalar.activation(
                        E4[:, :, rt, 1 : N + 1],
                        p2[:].rearrange("p (k w) -> p k w", w=N),
                        AF.Abs,
                        scale=s,
                    )
                    # u: gx-slice += 2*gy -> 2*(gx+gy); P = (s/sqrt2)*|2u|
                    #   mneg test: |u| < sqrt2|gx| <=> P < A
                    _accum_mm(nc, p2, [sl("y", c) for c in range(NCHUNK)],
                              w_sb["msm"], stop=True, out_off=N,
                              skip_group_check=True)
                    nc.scalar.activation(
                        Pq[:, rt * N : (rt + 1) * N],
                        p2[:, N : 2 * N],
                        AF.Abs,
                        scale=s / RT2,
                    )

                # ---------- Up/Dn shifted copies of the S plane via DMA ------
                # (issued per half so the select chain starts before the whole
                # image's pass-2 finishes)
                Upq = post.tile([128, NCHUNK * W2], F16, tag="Up")
                Dnq = post.tile([128, NCHUNK * W2], F16, tag="Dn")
                S3 = E4[:, 0]  # [128, 4, 514]
                U3 = Upq[:].rearrange("p (q w) -> p q w", w=W2)
                D3 = Dnq[:].rearrange("p (q w) -> p q w", w=W2)
                # boundary zeros (image edge rows): Up[127, 3] = 0; Dn[0, 0] = 0
                # (engine ops need quadrant-aligned start partitions; the
                # partition-127 sliver goes via DMA from the zero row)
                nc.sync.dma_start(
                    out=U3[127:128, 3:4, :],
                    in_=zrow[:].rearrange("p (q w) -> p q w", w=W2),
                )
                nc.gpsimd.memset(D3[0:1, 0:1, :], 0.0)

                A3 = E4[:, 1, :, 1 : N + 1]   # 4s|gx|
                Y3v = E4[:, 2, :, 1 : N + 1]  # 4s|gy|
                P3 = Pq[:].rearrange("p (q w) -> p q w", w=N)

                def qt(pool, tag, dt=F16):
                    t = pool.tile([128, NCHUNK * N], dt, tag=tag)
                    return t, t[:].rearrange("p (q w) -> p q w", w=N)

                # Y plane is extracted as (4s/t1)|gy| (scale folded into the
                # pass-1 weights), so m0 = is_ge(A, Y) directly; m2 needs
                # Yhi = t1^2 * Y = 4s*t1*|gy|
                Yhiq, Yhi3 = qt(post1, "Yhi")
                m0q, m03 = qt(post1, "m0", U16)
                m2q, m23 = qt(post1, "m2", U16)
                mnq, mn3 = qt(post1, "mn", U16)
                selq, sel3 = qt(post, "sel")
                pmAq, pmA3 = qt(post, "pmA")
                pmVq, pmV3 = qt(post, "pmV")
                pmHq, pmH3 = qt(post, "pmH")
                Cq, C3 = qt(post1, "C")
                zq, z3 = qt(post1, "z")
                zrq, zr3 = qt(post1, "zr")
                oq = outp.tile([128, NCHUNK * N], F16, tag="o")
                o3 = oq[:].rearrange("p (q w) -> p q w", w=N)
                x3 = xh[i].rearrange("p (q w) -> p q w", w=N)

                TTv = nc.vector.tensor_tensor
                TTp = nc.gpsimd.tensor_tensor
                TSv = nc.vector.tensor_scalar

                # the select stage runs per image-half: shorter dependency
                # chains pipeline better across engines; the last image's
                # second half runs as quarters to shorten the drain tail
                if i == 0:
                    parts = [(0, 1), (1, 2), (2, 4)]
                else:
                    parts = [(0, 2), (2, 4)]
                for q0, q1 in parts:
                    hs = slice(q0, q1)
                    # Up/Dn: main shift + wrap rows
                    nc.sync.dma_start(out=U3[0:127, hs, :],
                                      in_=S3[1:128, hs, :])
                    nc.sync.dma_start(out=D3[1:128, hs, :],
                                      in_=S3[0:127, hs, :])
                    # Up[127, q] = S[0, q+1] (q < 3)
                    qh = min(q1, 3)
                    if qh > q0:
                        nc.sync.dma_start(out=U3[127:128, q0:qh, :],
                                          in_=S3[0:1, q0 + 1 : qh + 1, :])
                    # Dn[0, q] = S[127, q-1] (q > 0)
                    ql = max(q0, 1)
                    if q1 > ql:
                        nc.sync.dma_start(out=D3[0:1, ql:q1, :],
                                          in_=S3[127:128, ql - 1 : q1 - 1, :])

                    # scaled |gy| copy (DVE tensor_scalar, 4x mode)
                    TSv(out=Yhi3[:, hs], in0=Y3v[:, hs], scalar1=T1 * T1,
                        op0=ALU.mult, scalar2=None)
                    # masks (DVE tensor_tensor, 2x mode)
                    TTv(out=m03[:, hs], in0=A3[:, hs], in1=Y3v[:, hs],
                        op=ALU.is_ge)
                    TTv(out=m23[:, hs], in0=Yhi3[:, hs], in1=A3[:, hs],
                        op=ALU.is_gt)
                    TTv(out=mn3[:, hs], in0=A3[:, hs], in1=P3[:, hs],
                        op=ALU.is_gt)
                    # pair maxes (DVE: Pool's GPSIMD has no max kernel)
                    TTv(out=sel3[:, hs], in0=D3[:, hs, 0:N],
                        in1=U3[:, hs, 2 : N + 2], op=ALU.max)  # main diag
                    TTv(out=pmA3[:, hs], in0=D3[:, hs, 2 : N + 2],
                        in1=U3[:, hs, 0:N], op=ALU.max)  # anti diag
                    TTv(out=pmV3[:, hs], in0=D3[:, hs, 1 : N + 1],
                        in1=U3[:, hs, 1 : N + 1], op=ALU.max)  # vertical
                    TTv(out=pmH3[:, hs], in0=S3[:, hs, 0:N],
                        in1=S3[:, hs, 2 : N + 2], op=ALU.max)  # horizontal
                    # selection cascade (DVE copy_predicated)
                    nc.vector.copy_predicated(sel3[:, hs], mn3[:, hs],
                                              pmA3[:, hs])
                    nc.vector.copy_predicated(sel3[:, hs], m23[:, hs],
                                              pmV3[:, hs])
                    nc.vector.copy_predicated(sel3[:, hs], m03[:, hs],
                                              pmH3[:, hs])
                    # z = zs*S - sel ; out = relu(z) * x
                    # Pool runs this chain (ts_mult/tt_sub/ts_max/tt_mult are
                    # the GPSIMD kernels that exist); last image on DVE for a
                    # short drain tail.
                    TSx = TSv if i == B_CORE - 1 else nc.gpsimd.tensor_scalar
                    TTc = TTv if i == B_CORE - 1 else TTp
                    # C is off the select critical path: keep it on Pool even
                    # for the last image so it overlaps the cp cascade
                    nc.gpsimd.tensor_scalar(
                        out=C3[:, hs], in0=S3[:, hs, 1 : N + 1], scalar1=zs,
                        op0=ALU.mult, scalar2=None)
                    TTc(out=z3[:, hs], in0=C3[:, hs], in1=sel3[:, hs],
                        op=ALU.subtract)
                    TSx(out=zr3[:, hs], in0=z3[:, hs], scalar1=0.0,
                        op0=ALU.max, scalar2=None)
                    TTc(out=o3[:, hs], in0=zr3[:, hs], in1=x3[:, hs],
                        op=ALU.mult)
                    # store this part (fp16 -> fp32 cast)
                    nc.gpsimd.dma_start(
                        out=out_d[i * N + 128 * q0 : i * N + 128 * q1, :]
                        .rearrange("(r p) w -> p r w", p=128),
                        in_=o3[:, hs],
                    )

    nc.compile()
    return nc


# ---------------------------------------------------------------------------
# host side
# ---------------------------------------------------------------------------

def _make_band(weights, offsets, pad):
    M = np.zeros((N, N), dtype=np.float64)
    for w, o in zip(weights, offsets):
        idx = np.arange(N)
        src = idx + o
        if pad == "replicate":
            np.add.at(M, (np.clip(src, 0, N - 1), idx), w)
        else:
            ok = (src >= 0) & (src < N)
            np.add.at(M, (src[ok], idx[ok]), w)
    return M


def _host_weights(gauss_kernel):
    gk = np.asarray(gauss_kernel, dtype=np.float64)[0, 0]
    U, sv, Vt = np.linalg.svd(gk)
    assert sv[1] < 1e-5 * sv[0], "gauss kernel not rank-1 separable"
    wv = U[:, 0] * np.sqrt(sv[0])
    wh = Vt[0] * np.sqrt(sv[0])
    if wv.sum() < 0:
        wv, wh = -wv, -wh
    # symmetry of the gauss kernel => Bv == Bh (same 1-D factor both ways)
    assert np.allclose(wv, wh, atol=1e-12), "gauss kernel not symmetric"
    o5 = [-2, -1, 0, 1, 2]
    o3 = [-1, 0, 1]
    G = _make_band(wv, o5, "zero")
    Sm = _make_band([1, 2, 1], o3, "replicate")
    Df = _make_band([-1, 0, 1], o3, "replicate")
    full = [G, 2.0 * (G @ Sm), 2.0 * (G @ Df),
            (2.0 / T1) * (G @ Sm)]
    # compact band strips: per (kernel, row-chunk) only cols R7[r] are read
    strips = np.zeros((4, 4, 128, 136), dtype=np.float64)
    for k in range(4):
        for r in range(4):
            lo = max(0, 128 * r - 3)
            hi = min(512, 128 * r + 131)
            strips[k, r, :, : hi - lo] = full[k][128 * r : 128 * (r + 1), lo:hi]
    return {"wcat": np.ascontiguousarray(
        strips.reshape(4 * 4 * 128, 136), dtype=np.float16)}



_NC_CACHE = {}
LAST_RESULT = None


def kernel(reconst, gauss_kernel, nms_kernel):
    nk = np.asarray(nms_kernel, dtype=np.float64)
    cen = float(nk[0, 0, 1, 1])
    v = float(nk[0, 0, 1, 2])
    # verify nms kernel structure: center + single tap v per direction
    pos = [(1, 2), (2, 2), (2, 1), (2, 0), (1, 0), (0, 0), (0, 1), (0, 2)]
    for d, (r, c) in enumerate(pos):
        k = nk[d, 0].copy()
        assert abs(k[1, 1] - cen) < 1e-6 and abs(k[r, c] - v) < 1e-6
        k[1, 1] = 0.0
        k[r, c] = 0.0
        assert np.abs(k).max() < 1e-7
    assert v < 0

    key = (round(cen, 9), round(v, 9))
    if key not in _NC_CACHE:
        _NC_CACHE[key] = build_nc(cen, v)
    nc = _NC_CACHE[key]

    w = _host_weights(gauss_kernel)
    x = np.asarray(reconst, dtype=np.float32).reshape(B_TOTAL, N, N)
    in_maps = []
    for core in range(N_CORES):
        m = {"x": np.ascontiguousarray(
            x[core * B_CORE : (core + 1) * B_CORE].reshape(B_CORE * N, N)
        )}
        m.update(w)
        in_maps.append(m)

    res = run_bass_kernel_spmd(nc, in_maps, core_ids=list(range(N_CORES)))
    global LAST_RESULT
    LAST_RESULT = res
    out = np.concatenate(
        [r["out"].reshape(B_CORE, 1, N, N) for r in res.results], axis=0
    )
    return out.astype(np.float32)


# revision 83
# speedup vs baseline: 1.0498x; 1.0048x over previous
"""Trainium2 Bass kernel for nn_Densenet_with_skip (gauss blur -> sobel ->
angle-binned 8-direction NMS -> gate).

Reformulation (same math as the validated baseline):
  b  = gauss5x5(x)                      (zero pad; separable, rank-1)
  gx/gy via composed 7-tap band matmuls (replicate pad on b)
  bin: m0 = (t1*|gx| >= |gy|)  -> horizontal pair
       m2 = (t1*|gy| >  |gx|)  -> vertical pair
       else diag: gx*gy<0 (<=> |gx+gy| < sqrt2*|gx|) -> anti-diag pair
  out = x * relu(cen*b + v*max(pair))

v2 engine-balance rewrite (vs the STT-heavy baseline):
  - 3 deduped weight matrices (gauss is symmetric: Bv==Bh; Sv==Sh; Dv==Dh),
    shared between both matmul passes; scales folded into the pass-1/pass-2
    PSUM->SBUF extract `scale` so every elementwise compare is a plain
    fp16 tensor_tensor (2x DVE mode) or tensor_scalar (4x DVE mode).
  - pass-2 accumulates [b | gx | gy] into ONE 3-bank PSUM tile per row-tile
    so a single Abs-activation extracts S/A/Y together (b >= 0 so Abs is a
    no-op on the b slice); |gx+gy| rides the same psum via 4 extra matmuls.
  - engine split: masks/pair-maxes/selection (copy_predicated, u16 masks)
    on DVE; the zs*S-sel / relu / gate chain on Pool (only add/sub/mult/
    ts_mult/ts_max exist as GPSIMD kernels; max/is_ge/STT do not compile);
    PSUM->SBUF extracts on Act; last image's chain on DVE for a short tail.
  - per-image-half select stage for cross-engine pipelining; batched DMAs
    (x load / out store); Up/Dn row-shifted copies of S on the DMA engines.
"""

import sys

import numpy as np

sys.path.insert(0, "/opt/trn_rl_repo")

import concourse.bacc as bacc
import concourse.mybir as mybir
from concourse import tile
from concourse.bass_utils import run_bass_kernel_spmd

N = 512
B_TOTAL = 32
N_CORES = 8
B_CORE = B_TOTAL // N_CORES  # 4 images per core
NCHUNK = N // 128  # 4

F16 = mybir.dt.float16
F32 = mybir.dt.float32
U16 = mybir.dt.uint16

T1 = float(np.tan(np.pi / 8))  # tan(22.5 deg)
RT2 = float(np.sqrt(2.0))

ALU = mybir.AluOpType
AF = mybir.ActivationFunctionType


def _band_ranges(halo):
    out = []
    for r in range(NCHUNK):
        lo = max(0, 128 * r - halo)
        hi = min(N, 128 * r + 128 + halo)
        out.append((lo, hi))
    return out


R7 = _band_ranges(3)
WSTRIP = 136  # compact W strip width (>= max band range width 134)


def _banded_mm(nc, psum_ap, lhsT_sl, w_chunks, start_fresh=True, stop=True,
               out_off=0):
    """Accumulate sum_r lhsT_r.T @ W_r[:, band] into psum[:, out_off + band]
    with PSUM fresh/covered split handling (each matmul touches either
    all-fresh or all-covered columns)."""
    covered = 0
    n = NCHUNK
    for r in range(n):
        lo, hi = R7[r]
        first = r == 0
        last = r == n - 1
        base = R7[r][0]
        if not first and lo < covered:
            nc.tensor.matmul(
                psum_ap[:, out_off + lo : out_off + covered],
                lhsT_sl[r],
                w_chunks[:, r, lo - base : covered - base],
                start=False,
                stop=False,
            )
            lo = covered
        nc.tensor.matmul(
            psum_ap[:, out_off + lo : out_off + hi],
            lhsT_sl[r],
            w_chunks[:, r, lo - base : hi - base],
            start=first and start_fresh,
            stop=last and stop,
        )
        covered = hi


def _accum_mm(nc, psum_ap, lhsT_sl, w_chunks, stop=True, out_off=0,
              skip_group_check=False):
    """Accumulate onto an already-covered psum range (no splits needed)."""
    for r in range(NCHUNK):
        lo, hi = R7[r]
        base = R7[r][0]
        nc.tensor.matmul(
            psum_ap[:, out_off + lo : out_off + hi],
            lhsT_sl[r],
            w_chunks[:, r, lo - base : hi - base],
            start=False,
            stop=stop and (r == NCHUNK - 1),
            skip_group_check=skip_group_check,
        )


def build_nc(cen, v):
    s = -v  # S = s*b ; z = (cen/s)*S - sel
    zs = cen / s

    nc = bacc.Bacc("TRN2", target_bir_lowering=False, debug=False)

    x_d = nc.dram_tensor("x", [B_CORE * N, N], F32, kind="ExternalInput").ap()
    # three deduped band matrices as one cat tensor [3, 512, 512] fp16:
    # plane 0: G (gauss 5-tap), plane 1: 2*M_sm (Bv@Sm), plane 2: 2*M_df
    wcat_d = nc.dram_tensor(
        "wcat", [4 * NCHUNK * 128, WSTRIP], F16, kind="ExternalInput"
    ).ap()
    out_d = nc.dram_tensor("out", [B_CORE * N, N], F32, kind="ExternalOutput").ap()

    W2 = N + 2  # padded row width for col-shifted access

    with tile.TileContext(nc) as tc:
        with (
            tc.tile_pool(name="wpool", bufs=1) as wpool,
            tc.tile_pool(name="xpool", bufs=1) as xpool,
            tc.tile_pool(name="tT", bufs=2) as tTpool,
            tc.tile_pool(name="post", bufs=2) as post,
            tc.tile_pool(name="post1", bufs=1) as post1,
            tc.tile_pool(name="outp", bufs=2) as outp,
            tc.tile_pool(name="psum", bufs=2, space="PSUM") as psum,
        ):
            zrow = wpool.tile([1, W2], F16, tag="zrow")
            nc.vector.memset(zrow[:], 0.0)

            # --- load x as fp16 (DMA cast); quad layout [128, 4*512]:
            # image 0 alone (so compute starts early), images 1-3 batched
            x0t = xpool.tile([128, NCHUNK * N], F16, tag="xh_0")
            nc.gpsimd.dma_start(
                out=x0t[:].rearrange("p (r w) -> p r w", w=N),
                in_=x_d[0:N, :].rearrange("(r p) w -> p r w", p=128),
            )
            xh = [x0t[:], None, None, None]

            # --- weight cat [128, 3, 4, 512]; one DMA per row-chunk so the
            # first pass-1 matmuls can start early
            wt = wpool.tile([128, 4 * NCHUNK * WSTRIP], F16, tag="wcat")
            w4 = wt[:].rearrange("p (k r w) -> p k r w", k=4, w=WSTRIP)
            wsrc = wcat_d.rearrange("(k r p) w -> p r k w", k=4, p=128)
            for r in range(NCHUNK):
                nc.sync.dma_start(out=w4[:, :, r, :], in_=wsrc[:, r])
            # plane 3 = t1*2*M_sm: the u-sweep needs gy at the SAME scale as
            # gx even though t_y carries an extra 1/t1 for the m0 threshold
            w_sb = {"g": w4[:, 0], "msm": w4[:, 1], "mdf": w4[:, 2],
                    "msmt": w4[:, 3]}

            # images 1-3 in one DMA (overlaps image-0 compute)
            x13 = xpool.tile([128, 3 * NCHUNK * N], F16, tag="xh_13")
            nc.gpsimd.dma_start(
                out=x13[:].rearrange("p (i r w) -> p i r w", i=3, w=N),
                in_=x_d[N:, :].rearrange("(i r p) w -> p i r w", p=128, i=3),
            )
            for i in range(1, B_CORE):
                xh[i] = x13[:, (i - 1) * NCHUNK * N : i * NCHUNK * N]

            for i in range(B_CORE):
                # ---------- pass 1 (vertical), all 3 kernels per matmul ------
                # psum planes: [G | 2*M_sm | 2*M_df] applied down the rows;
                # one plain Copy extract per col-chunk (scales live in W).
                tTc = []
                for c in range(NCHUNK):
                    pv = psum.tile([128, 3 * N], F32, tag="cat")
                    pv3 = pv[:].rearrange("p (k w) -> p k w", w=N)
                    lhsT = [
                        xh[i][:, N * r + 128 * c : N * r + 128 * (c + 1)]
                        for r in range(NCHUNK)
                    ]
                    # one matmul per kernel-plane: a psum AP must stay inside
                    # a single 2KB psum bank
                    covered = 0
                    for r in range(NCHUNK):
                        lo, hi = R7[r]
                        first = r == 0
                        base = R7[r][0]
                        if not first and lo < covered:
                            for k in range(3):
                                nc.tensor.matmul(
                                    pv3[:, k, lo:covered], lhsT[r],
                                    w4[:, k, r, lo - base : covered - base],
                                    start=False, stop=False,
                                )
                            lo = covered
                        for k in range(3):
                            nc.tensor.matmul(
                                pv3[:, k, lo:hi], lhsT[r],
                                w4[:, k, r, lo - base : hi - base],
                                start=first, stop=(r == NCHUNK - 1),
                            )
                        covered = hi
                    st = tTpool.tile([128, 3 * N], F16, tag=f"tT_{c}")
                    if i == 0 and c % 2 == 0:
                        # during pipeline fill DVE is idle: split image 0's
                        # extracts between Act and DVE to start sooner
                        nc.vector.tensor_scalar(out=st[:], in0=pv[:],
                                                scalar1=1.0, op0=ALU.mult,
                                                scalar2=None)
                    else:
                        nc.scalar.activation(st[:], pv[:], AF.Copy)
                    tTc.append(st)
                # pass-2 stationary slices: tTc[c][:, k*512 + rows]
                tT = {
                    k: [tTc[c][:, kk * N : (kk + 1) * N] for c in range(NCHUNK)]
                    for kk, k in enumerate(("b", "x", "y"))
                }

                # ---------- pass 2 (horizontal) into cat psum [128, 3*512] ----
                # slice 0: b ; slice 1: 2*gx (then 2*(gx+gy)) ; slice 2: 2*gy
                # Eq holds the Abs-extract [S | A | Y] in padded quad layout
                # [128, 3, 4, 514]: S = s*b, A = 2s|gx|, Y = 2s|gy|
                Eq = post.tile([128, 3 * NCHUNK * W2], F16, tag="Eq")
                E4 = Eq[:].rearrange("p (k q w) -> p k q w", k=3, w=W2)
                # zero the S-plane column pads (cols 0 and 513 of each q)
                nc.gpsimd.memset(E4[:, 0, :, 0:1], 0.0)
                nc.gpsimd.memset(E4[:, 0, :, N + 1 : N + 2], 0.0)
                Pq = post.tile([128, NCHUNK * N], F16, tag="Pq")

                for rt in range(NCHUNK):
                    row0 = 128 * rt
                    p2 = psum.tile([128, 3 * N], F32, tag="cat")

                    def sl(key, c):
                        return tT[key][c][:, row0 : row0 + 128]

                    _banded_mm(nc, p2, [sl("b", c) for c in range(NCHUNK)],
                               w_sb["g"], stop=True, out_off=0)
                    _banded_mm(nc, p2, [sl("x", c) for c in range(NCHUNK)],
                               w_sb["mdf"], stop=True, out_off=N)
                    _banded_mm(nc, p2, [sl("y", c) for c in range(NCHUNK)],
                               w_sb["msmt"], stop=True, out_off=2 * N)

                    # one Abs extract of all 3 slices; scale s gives
                    # S = s|b| = s*b, A = 2s|gx|, Y = 2s|gy|
                    nc.scalar.activation(
                        E4[:, :, rt, 1 : N + 1],
                        p2[:].rearrange("p (k w) -> p k w", w=N),
                        AF.Abs,
                        scale=s,
                    )
                    # u: gx-slice += 2*gy -> 2*(gx+gy); P = (s/sqrt2)*|2u|
                    #   mneg test: |u| < sqrt2|gx| <=> P < A
                    _accum_mm(nc, p2, [sl("y", c) for c in range(NCHUNK)],
                              w_sb["msm"], stop=True, out_off=N,
                              skip_group_check=True)
                    nc.scalar.activation(
                        Pq[:, rt * N : (rt + 1) * N],
                        p2[:, N : 2 * N],
                        AF.Abs,
                        scale=s / RT2,
                    )

                # ---------- Up/Dn shifted copies of the S plane via DMA ------
                # (issued per half so the select chain starts before the whole
                # image's pass-2 finishes)
                Upq = post.tile([128, NCHUNK * W2], F16, tag="Up")
                Dnq = post.tile([128, NCHUNK * W2], F16, tag="Dn")
                S3 = E4[:, 0]  # [128, 4, 514]
                U3 = Upq[:].rearrange("p (q w) -> p q w", w=W2)
                D3 = Dnq[:].rearrange("p (q w) -> p q w", w=W2)
                # boundary zeros (image edge rows): Up[127, 3] = 0; Dn[0, 0] = 0
                # (engine ops need quadrant-aligned start partitions; the
                # partition-127 sliver goes via DMA from the zero row)
                nc.sync.dma_start(
                    out=U3[127:128, 3:4, :],
                    in_=zrow[:].rearrange("p (q w) -> p q w", w=W2),
                )
                nc.gpsimd.memset(D3[0:1, 0:1, :], 0.0)

                A3 = E4[:, 1, :, 1 : N + 1]   # 4s|gx|
                Y3v = E4[:, 2, :, 1 : N + 1]  # 4s|gy|
                P3 = Pq[:].rearrange("p (q w) -> p q w", w=N)

                def qt(pool, tag, dt=F16):
                    t = pool.tile([128, NCHUNK * N], dt, tag=tag)
                    return t, t[:].rearrange("p (q w) -> p q w", w=N)

                # Y plane is extracted as (4s/t1)|gy| (scale folded into the
                # pass-1 weights), so m0 = is_ge(A, Y) directly; m2 needs
                # Yhi = t1^2 * Y = 4s*t1*|gy|
                Yhiq, Yhi3 = qt(post1, "Yhi")
                m0q, m03 = qt(post1, "m0", U16)
                m2q, m23 = qt(post1, "m2", U16)
                mnq, mn3 = qt(post1, "mn", U16)
                selq, sel3 = qt(post, "sel")
                pmAq, pmA3 = qt(post, "pmA")
                pmVq, pmV3 = qt(post, "pmV")
                pmHq, pmH3 = qt(post, "pmH")
                Cq, C3 = qt(post1, "C")
                zq, z3 = qt(post1, "z")
                zrq, zr3 = qt(post1, "zr")
                oq = outp.tile([128, NCHUNK * N], F16, tag="o")
                o3 = oq[:].rearrange("p (q w) -> p q w", w=N)
                x3 = xh[i].rearrange("p (q w) -> p q w", w=N)

                TTv = nc.vector.tensor_tensor
                TTp = nc.gpsimd.tensor_tensor
                TSv = nc.vector.tensor_scalar

                # the select stage runs per image-half: shorter dependency
                # chains pipeline better across engines; the last image's
                # second half runs as quarters to shorten the drain tail
                if i == 0:
                    parts = [(0, 1), (1, 2), (2, 4)]
                elif i == 1:
                    parts = [(0, 1), (1, 2), (2, 4)]
                else:
                    parts = [(0, 2), (2, 4)]
                for q0, q1 in parts:
                    hs = slice(q0, q1)
                    # Up/Dn: main shift + wrap rows
                    nc.sync.dma_start(out=U3[0:127, hs, :],
                                      in_=S3[1:128, hs, :])
                    nc.sync.dma_start(out=D3[1:128, hs, :],
                                      in_=S3[0:127, hs, :])
                    # Up[127, q] = S[0, q+1] (q < 3)
                    qh = min(q1, 3)
                    if qh > q0:
                        nc.sync.dma_start(out=U3[127:128, q0:qh, :],
                                          in_=S3[0:1, q0 + 1 : qh + 1, :])
                    # Dn[0, q] = S[127, q-1] (q > 0)
                    ql = max(q0, 1)
                    if q1 > ql:
                        nc.sync.dma_start(out=D3[0:1, ql:q1, :],
                                          in_=S3[127:128, ql - 1 : q1 - 1, :])

                    # scaled |gy| copy (DVE tensor_scalar, 4x mode)
                    TSv(out=Yhi3[:, hs], in0=Y3v[:, hs], scalar1=T1 * T1,
                        op0=ALU.mult, scalar2=None)
                    # masks (DVE tensor_tensor, 2x mode)
                    TTv(out=m03[:, hs], in0=A3[:, hs], in1=Y3v[:, hs],
                        op=ALU.is_ge)
                    TTv(out=m23[:, hs], in0=Yhi3[:, hs], in1=A3[:, hs],
                        op=ALU.is_gt)
                    TTv(out=mn3[:, hs], in0=A3[:, hs], in1=P3[:, hs],
                        op=ALU.is_gt)
                    # pair maxes (DVE: Pool's GPSIMD has no max kernel)
                    TTv(out=sel3[:, hs], in0=D3[:, hs, 0:N],
                        in1=U3[:, hs, 2 : N + 2], op=ALU.max)  # main diag
                    TTv(out=pmA3[:, hs], in0=D3[:, hs, 2 : N + 2],
                        in1=U3[:, hs, 0:N], op=ALU.max)  # anti diag
                    TTv(out=pmV3[:, hs], in0=D3[:, hs, 1 : N + 1],
                        in1=U3[:, hs, 1 : N + 1], op=ALU.max)  # vertical
                    TTv(out=pmH3[:, hs], in0=S3[:, hs, 0:N],
                        in1=S3[:, hs, 2 : N + 2], op=ALU.max)  # horizontal
                    # selection cascade (DVE copy_predicated)
                    nc.vector.copy_predicated(sel3[:, hs], mn3[:, hs],
                                              pmA3[:, hs])
                    nc.vector.copy_predicated(sel3[:, hs], m23[:, hs],
                                              pmV3[:, hs])
                    nc.vector.copy_predicated(sel3[:, hs], m03[:, hs],
                                              pmH3[:, hs])
                    # z = zs*S - sel ; out = relu(z) * x
                    # Pool runs this chain (ts_mult/tt_sub/ts_max/tt_mult are
                    # the GPSIMD kernels that exist); last image on DVE for a
                    # short drain tail.
                    TSx = TSv if i == B_CORE - 1 else nc.gpsimd.tensor_scalar
                    TTc = TTv if i == B_CORE - 1 else TTp
                    # C is off the select critical path: keep it on Pool even
                    # for the last image so it overlaps the cp cascade
                    nc.gpsimd.tensor_scalar(
                        out=C3[:, hs], in0=S3[:, hs, 1 : N + 1], scalar1=zs,
                        op0=ALU.mult, scalar2=None)
                    TTc(out=z3[:, hs], in0=C3[:, hs], in1=sel3[:, hs],
                        op=ALU.subtract)
                    TSx(out=zr3[:, hs], in0=z3[:, hs], scalar1=0.0,
                        op0=ALU.max, scalar2=None)
                    TTc(out=o3[:, hs], in0=zr3[:, hs], in1=x3[:, hs],
                        op=ALU.mult)
                    # store this part (fp16 -> fp32 cast)
                    nc.gpsimd.dma_start(
                        out=out_d[i * N + 128 * q0 : i * N + 128 * q1, :]
                        .rearrange("(r p) w -> p r w", p=128),
                        in_=o3[:, hs],
                    )

    nc.compile()
    return nc


# ---------------------------------------------------------------------------
# host side
# ---------------------------------------------------------------------------

def _make_band(weights, offsets, pad):
    M = np.zeros((N, N), dtype=np.float64)
    for w, o in zip(weights, offsets):
        idx = np.arange(N)
        src = idx + o
        if pad == "replicate":
            np.add.at(M, (np.clip(src, 0, N - 1), idx), w)
        else:
            ok = (src >= 0) & (src < N)
            np.add.at(M, (src[ok], idx[ok]), w)
    return M


def _host_weights(gauss_kernel):
    gk = np.asarray(gauss_kernel, dtype=np.float64)[0, 0]
    U, sv, Vt = np.linalg.svd(gk)
    assert sv[1] < 1e-5 * sv[0], "gauss kernel not rank-1 separable"
    wv = U[:, 0] * np.sqrt(sv[0])
    wh = Vt[0] * np.sqrt(sv[0])
    if wv.sum() < 0:
        wv, wh = -wv, -wh
    # symmetry of the gauss kernel => Bv == Bh (same 1-D factor both ways)
    assert np.allclose(wv, wh, atol=1e-12), "gauss kernel not symmetric"
    o5 = [-2, -1, 0, 1, 2]
    o3 = [-1, 0, 1]
    G = _make_band(wv, o5, "zero")
    Sm = _make_band([1, 2, 1], o3, "replicate")
    Df = _make_band([-1, 0, 1], o3, "replicate")
    full = [G, 2.0 * (G @ Sm), 2.0 * (G @ Df),
            (2.0 / T1) * (G @ Sm)]
    # compact band strips: per (kernel, row-chunk) only cols R7[r] are read
    strips = np.zeros((4, 4, 128, 136), dtype=np.float64)
    for k in range(4):
        for r in range(4):
            lo = max(0, 128 * r - 3)
            hi = min(512, 128 * r + 131)
            strips[k, r, :, : hi - lo] = full[k][128 * r : 128 * (r + 1), lo:hi]
    return {"wcat": np.ascontiguousarray(
        strips.reshape(4 * 4 * 128, 136), dtype=np.float16)}



_NC_CACHE = {}
LAST_RESULT = None


def kernel(reconst, gauss_kernel, nms_kernel):
    nk = np.asarray(nms_kernel, dtype=np.float64)
    cen = float(nk[0, 0, 1, 1])
    v = float(nk[0, 0, 1, 2])
    # verify nms kernel structure: center + single tap v per direction
    pos = [(1, 2), (2, 2), (2, 1), (2, 0), (1, 0), (0, 0), (0, 1), (0, 2)]
    for d, (r, c) in enumerate(pos):
        k = nk[d, 0].copy()
        assert abs(k[1, 1] - cen) < 1e-6 and abs(k[r, c] - v) < 1e-6
        k[1, 1] = 0.0
        k[r, c] = 0.0
        assert np.abs(k).max() < 1e-7
    assert v < 0

    key = (round(cen, 9), round(v, 9))
    if key not in _NC_CACHE:
        _NC_CACHE[key] = build_nc(cen, v)
    nc = _NC_CACHE[key]

    w = _host_weights(gauss_kernel)
    x = np.asarray(reconst, dtype=np.float32).reshape(B_TOTAL, N, N)
    in_maps = []
    for core in range(N_CORES):
        m = {"x": np.ascontiguousarray(
            x[core * B_CORE : (core + 1) * B_CORE].reshape(B_CORE * N, N)
        )}
        m.update(w)
        in_maps.append(m)

    res = run_bass_kernel_spmd(nc, in_maps, core_ids=list(range(N_CORES)))
    global LAST_RESULT
    LAST_RESULT = res
    out = np.concatenate(
        [r["out"].reshape(B_CORE, 1, N, N) for r in res.results], axis=0
    )
    return out.astype(np.float32)
